# revision 31
# baseline (speedup 1.0000x reference)
"""H2GCN neighborhood aggregation on 8 Trainium2 NeuronCores.

Computes concat([adj_t @ x, adj_t2 @ x], axis=1) for
adj_t/adj_t2: [8192, 8192] f32, x: [8192, 256] f32.

Sharding: row-shard adj_t/adj_t2 (1024 rows per core), replicate x,
each core produces its [1024, 512] slice of the output.

fp8v2 mode (default): adjacency is centered (a - 0.5) and quantized to
fp8 e4m3 on host, x quantized to e4m3, and the rank-1 term
0.5 * colsum(x) is carried exactly in f32 and added after accumulation.
Matmuls run in MatmulPerfMode.DoubleRow (256 contraction rows per
instruction, 2x PE throughput); measured ~153 TFLOP/s/core, ~98% of the
fp8 PE peak. Adjacency HBM traffic is halved vs bf16 so DMA (~41 us)
hides fully under the PE stream (~56 us). Measured rel err vs the f32
reference: 1.4e-2 (gate 2e-2).

Per-core dataflow (fp8v2):
  - host packs the adjacency slice p-major [128p, 32kk, 2, 1024m] e4m3
    (64 KB contiguous per partition) and x as [128p, 32kk, 2, 256d], so
    every DMA descriptor is a single >=2KB contiguous run.
  - adjacency streams in CKK-kk chunks on the sync HWDGE queue; x lands
    in staged per-group tiles (first tiny group leads the sync ring so
    the first matmul un-gates at ~2.5us; tile-granular dep tracking
    would otherwise gate every matmul on the last x DMA). chunk k's
    4*CKK DoubleRow matmuls accumulate out^T blocks [128d, 512m] in the
    8 PSUM banks (both matrices resident, so the PE never drains
    between matrices).
  - a few throwaway matmuls at t=0 lift the PE HAM clock gate (cold PE
    runs at 1.2 GHz for ~3.4 us) while the first DMAs are in flight.
  - mat0's first / mat1's last chunk are split into single-kk pieces to
    shorten the DMA-gated head and the accumulate->copy->store tail.
  - DVE/Act add the f32 colsum bias during the PSUM -> SBUF bf16 copy;
    stores are per-(mat, d-block) [128, 1024] bf16 rows, the very last
    on the otherwise-idle sync queue. Host un-transposes and upcasts.
"""

import numpy as np

N = 8192
D = 256
CORES = 8
P = 128
M_LOC = N // CORES  # 1024 rows of each adjacency matrix per core
MB = M_LOC // P  # 8 output row-blocks per core
KB = N // P  # 64 contraction blocks
KK = N // (2 * P)  # 32 k-pair blocks (DoubleRow consumes 256 rows/step)
GRP = 8  # k-blocks per transpose/copy group (one PSUM bank), bf16t mode
N_GRP = KB // GRP  # 8

MODE = "fp8v2"  # "fp8v2" / "fp8drb" / "fp8dr" (e4m3 DoubleRow) / "bf16t"

TUNE = dict(
    a_bufs=12,  # fp8 chunk slots in flight
    xg=4,  # kk-blocks per x-load DMA
    chunk_kk=1,  # kk-pairs per adjacency chunk DMA
    out_bf16=True,  # store out as bf16, host upcasts
    diag="none",  # "nodma" (tiny chunk DMAs) / "nomm" (single matmul per acc)
    hwloop=True,  # use tc.For_i for the repeat loop (timing NEFFs only)
    il=True,  # host pair-interleaves adjacency rows (2KB contiguous/partition)
    v2_ckk=4,  # kk-pairs per adjacency chunk DMA (v2)
    v2_xg=8,  # kk-pairs per x-load DMA (v2)
    v2_abufs=8,  # adjacency chunk slots in flight (v2)
    v2_warm=24,  # dummy N=128 matmuls at t=0 to lift the HAM clock gate
    # (sized so PE activity is continuous from ~0.3us until the first
    # DMA-gated real matmul at ~2.9us — an idle gap would let the HAM's
    # free-running idle window re-throttle the clock)
    v2_2q=False,  # alternate chunk DMAs across both HWDGE rings
)

_cache = {}


def _build_fp8drb(repeat=1):
    """x-stationary orientation: out^T[d, m] accumulates in PSUM while the
    adjacency chunk streams as the 1024-wide moving operand (512 k-pairs).
    Stationary x blocks are reused across 2 moving streams, so LD_WEIGHTS
    is amortized 4x vs the a-stationary orientation. The colsum correction
    is a per-partition bias add; host un-transposes the [512, 1024] out.
    Output is stored bf16 (host upcasts); x loads stream on the gpsimd
    DMA queue interleaved with the first matrix's chunks."""
    import concourse.bacc as bacc
    import concourse.tile as tile
    import concourse.mybir as mybir

    F32 = mybir.dt.float32
    FP8 = mybir.dt.float8e4
    ODT = mybir.dt.bfloat16 if TUNE["out_bf16"] else F32
    DR = mybir.MatmulPerfMode.DoubleRow

    nc = bacc.Bacc(
        "TRN2",
        target_bir_lowering=False,
        debug=False,
        enable_asserts=False,
        num_devices=CORES,
    )
    at_ap = nc.dram_tensor("at", [N, M_LOC], FP8, kind="ExternalInput").ap()
    a2t_ap = nc.dram_tensor("a2t", [N, M_LOC], FP8, kind="ExternalInput").ap()
    x_ap = nc.dram_tensor("x", [N, D], FP8, kind="ExternalInput").ap()
    c_ap = nc.dram_tensor("c", [P, 2], F32, kind="ExternalInput").ap()
    # out is transposed: [2D, M_LOC]; host transposes back (and upcasts)
    out_ap = nc.dram_tensor("out", [2 * D, M_LOC], ODT, kind="ExternalOutput").ap()

    XG = TUNE["xg"]
    CKK = TUNE["chunk_kk"]  # kk-pairs per adjacency chunk DMA
    DBLK = D // P  # 2 stationary d-blocks
    MC = M_LOC // 512  # 2 moving chunks of 512 columns
    with tile.TileContext(nc) as tc:
        with (
            tc.tile_pool(name="xp", bufs=1) as x_pool,
            tc.tile_pool(name="cp", bufs=1) as c_pool,
            tc.tile_pool(name="ap", bufs=TUNE["a_bufs"]) as a_pool,
            tc.tile_pool(name="op", bufs=2 * DBLK * MC) as o_pool,
            tc.tile_pool(name="pacc", bufs=2 * DBLK * MC, space="PSUM") as acc_pool,
        ):
            c_t = c_pool.tile([P, 2], F32)
            nc.gpsimd.dma_start(c_t[:], c_ap[:])

            x_t = x_pool.tile([P, KK, 2, D], FP8)
            x_re = x_ap.rearrange("(kk i p) d -> p kk i d", p=P, i=2)

            for _rep in range(repeat):
                for mat, src_ap in ((0, at_ap), (1, a2t_ap)):
                    accs = [
                        acc_pool.tile([P, 512], F32, tag="acc", name=f"acc{i}")
                        for i in range(DBLK * MC)
                    ]
                    src_re = src_ap.rearrange("(kk i p) m -> p kk i m", p=P, i=2)
                    for kk0 in range(0, KK, CKK):
                        if _rep == 0 and mat == 0 and kk0 % XG == 0:
                            # x chunk rides the gpsimd queue, just ahead of
                            # the adjacency chunks that need it
                            nc.gpsimd.dma_start(
                                x_t[:, kk0 : kk0 + XG], x_re[:, kk0 : kk0 + XG]
                            )
                        ch = a_pool.tile([P, CKK, 2, M_LOC], FP8, tag="achunk")
                        nc.sync.dma_start(ch[:], src_re[:, kk0 : kk0 + CKK])
                        for j in range(CKK):
                            kk = kk0 + j
                            for db in range(DBLK):
                                for mc in range(MC):
                                    nc.tensor.matmul(
                                        accs[db * MC + mc][:],
                                        x_t[:, kk, :, db * P : (db + 1) * P],
                                        ch[:, j, :, mc * 512 : (mc + 1) * 512],
                                        start=(kk == 0),
                                        stop=(kk == KK - 1),
                                        perf_mode=DR,
                                    )
                    for db in range(DBLK):
                        for mc in range(MC):
                            ot = o_pool.tile([P, 512], ODT, tag="outt")
                            if (db * MC + mc) % 2 == 0:
                                nc.vector.tensor_scalar_add(
                                    ot[:], accs[db * MC + mc][:], c_t[:, db : db + 1]
                                )
                            else:
                                nc.scalar.add(
                                    ot[:], accs[db * MC + mc][:], c_t[:, db : db + 1]
                                )
                            nc.scalar.dma_start(
                                out_ap[
                                    mat * D + db * P : mat * D + (db + 1) * P,
                                    mc * 512 : (mc + 1) * 512,
                                ],
                                ot[:],
                            )

    nc.compile()
    return nc


def _emit_v2_out(nc, o_pool, accs, c_t, out_ap, mat, ODT):
    """PSUM -> SBUF bf16 copies (+colsum bias) and per-d-block out stores."""
    import concourse.mybir as mybir  # noqa: F401

    P_, D_, MC_ = P, D, 2
    for db in range(2):
        ot = o_pool.tile([P_, 2, 512], ODT, tag="outt")
        for mc in range(MC_):
            # one full-width copy per engine: splitting these across both
            # engines halves the per-acc drain but serializes 8 half-copies
            # plus store descriptor-gen on Act and sims 1.5us WORSE overall
            if mc == 0:
                nc.vector.tensor_scalar_add(
                    ot[:, mc], accs[db * MC_ + mc][:], c_t[:, db : db + 1]
                )
            else:
                nc.scalar.add(
                    ot[:, mc], accs[db * MC_ + mc][:], c_t[:, db : db + 1]
                )
        # mat0 stores must stay off sync so mat1's chunk stream is never
        # queued behind them (they're emitted before mat1's chunk loop);
        # mat1's stores ride the by-then-idle sync ring so their HWDGE
        # gens don't serialize with the Act-side tail copies
        rows = slice(mat * D_ + db * P_, mat * D_ + (db + 1) * P_)
        if mat == 1 and db == 1:
            # final store: split the two m-halves across both HWDGE rings
            # so the tail's last transfer is half-length and issues in
            # parallel as each copy lands
            nc.sync.dma_start(out_ap[rows, 0:512], ot[:, 0])
            nc.scalar.dma_start(out_ap[rows, 512:1024], ot[:, 1])
        elif mat == 1:
            nc.sync.dma_start(
                out_ap[rows, :], ot[:].rearrange("p mc m -> p (mc m)")
            )
        else:
            nc.scalar.dma_start(
                out_ap[rows, :], ot[:].rearrange("p mc m -> p (mc m)")
            )


def _build_fp8v2(repeat=1):
    """x-stationary orientation like fp8drb, with p-major DRAM layouts so
    every DMA descriptor is a single >=2KB contiguous run per partition:
      at/a2t: [P, KK, 2, M_LOC] fp8 (64 KB contiguous per partition),
      x:      [P, KK, 2, D]     fp8 (16 KB per partition).
    Queue plan (avoids prefetch-blocking): all adjacency chunks stream on
    sync (SP); x + c ride vector (DVE); PSUM->SBUF copies alternate
    vector/scalar; out stores are per-(mat, db) row-blocks [P, 2D] bf16 on
    scalar. Both matrices' accumulators live in the 8 PSUM banks so mat1
    matmuls start while mat0 drains."""
    import concourse.bacc as bacc
    import concourse.tile as tile
    import concourse.mybir as mybir

    F32 = mybir.dt.float32
    FP8 = mybir.dt.float8e4
    ODT = mybir.dt.bfloat16 if TUNE["out_bf16"] else F32
    DR = mybir.MatmulPerfMode.DoubleRow

    nc = bacc.Bacc(
        "TRN2",
        target_bir_lowering=False,
        debug=False,
        enable_asserts=False,
        num_devices=CORES,
    )
    at_ap = nc.dram_tensor("at", [P, KK, 2, M_LOC], FP8, kind="ExternalInput").ap()
    a2t_ap = nc.dram_tensor("a2t", [P, KK, 2, M_LOC], FP8, kind="ExternalInput").ap()
    x_ap = nc.dram_tensor("x", [P, KK, 2, D], FP8, kind="ExternalInput").ap()
    c_ap = nc.dram_tensor("c", [P, 2], F32, kind="ExternalInput").ap()
    # out is transposed: [2D, M_LOC]; host transposes back (and upcasts)
    out_ap = nc.dram_tensor("out", [2 * D, M_LOC], ODT, kind="ExternalOutput").ap()

    CKK = TUNE["v2_ckk"]  # kk-pairs per adjacency chunk DMA
    XG = TUNE["v2_xg"]  # kk-pairs per x DMA
    DBLK = D // P  # 2 stationary d-blocks
    MC = M_LOC // 512  # 2 moving chunks of 512 columns
    with tile.TileContext(nc) as tc:
        with (
            tc.tile_pool(name="xp", bufs=4) as x_pool,
            tc.tile_pool(name="cp", bufs=1) as c_pool,
            tc.tile_pool(name="ap", bufs=TUNE["v2_abufs"]) as a_pool,
            tc.tile_pool(name="op", bufs=2 * DBLK) as o_pool,
            tc.tile_pool(name="pacc", bufs=2 * DBLK * MC, space="PSUM") as acc_pool,
        ):
            if TUNE["v2_warm"]:
                # lift the PE HAM clock gate during the DMA ramp: a zeroed
                # fp8 tile feeds throwaway N=128 matmuls into acc bank 0;
                # the real accumulation's start=True reset makes them inert.
                # memset on vector so the gpsimd x-load queue isn't delayed.
                warm_t = c_pool.tile([P, 2, P], FP8)
                nc.vector.memset(warm_t[:], 0.0)

            # x arrives in staged groups, each its OWN tile: tile-granular
            # dependency tracking would otherwise gate every matmul on the
            # LAST x DMA (~+3.5us on the one-shot critical path). The first
            # tiny group is the very first transfer on the sync HWDGE ring
            # (~0.6us first-byte) so the first matmul un-gates immediately;
            # later groups land well before the PE stream reaches them.
            xgroups = [(0, 2), (2, 8), (10, 11), (21, KK - 21)]
            x_ts = {}
            for gi, (g0, gn) in enumerate(xgroups):
                xt = x_pool.tile([P, gn, 2, D], FP8, name=f"xg{gi}")
                q = nc.sync if gi == 0 else nc.gpsimd
                q.dma_start(xt[:], x_ap[:, g0 : g0 + gn])
                for kk in range(g0, g0 + gn):
                    x_ts[kk] = (xt, kk - g0)
            c_t = c_pool.tile([P, 2], F32)
            nc.gpsimd.dma_start(c_t[:], c_ap[:])

            # chunk plans: CKK-sized chunks; mat0's first block split fine so
            # the first matmul isn't gated on a large DMA, mat1's last block
            # split fine so the tail chain is short
            def mk_plan(head_split, tail_split):
                plan, kk0 = [], 0
                while kk0 < KK:
                    ck = min(CKK, KK - kk0)
                    fine = (head_split and kk0 == 0) or (
                        tail_split and kk0 + ck >= KK
                    )
                    if fine and ck > 1:
                        plan.extend((kk0 + j, 1) for j in range(ck))
                    else:
                        plan.append((kk0, ck))
                    kk0 += ck
                return plan

            if TUNE["v2_warm"]:
                warm_acc = acc_pool.tile([P, 512], F32, tag="acc", name="warm")
                for w in range(TUNE["v2_warm"]):
                    nc.tensor.matmul(
                        warm_acc[:, :128],
                        warm_t[:],
                        warm_t[:],
                        start=True,
                        stop=True,
                        perf_mode=DR,
                        skip_group_check=True,
                    )

            diag = TUNE["diag"]
            two_q = TUNE["v2_2q"]
            for _rep in range(repeat):
                all_accs = {}
                qi = 0
                for mat, src_ap in ((0, at_ap), (1, a2t_ap)):
                    plan = mk_plan(head_split=(_rep == 0 and mat == 0), tail_split=(mat == 1))
                    accs = [
                        acc_pool.tile([P, 512], F32, tag="acc", name=f"acc{mat}_{i}")
                        for i in range(DBLK * MC)
                    ]
                    all_accs[mat] = accs
                    for kk0, ck in plan:
                        ch = a_pool.tile([P, CKK, 2, M_LOC], FP8, tag="achunk")
                        q = nc.scalar if (two_q and qi % 2) else nc.sync
                        qi += 1
                        if diag == "nodma":
                            # 1/32 of the bytes: keeps the dep structure,
                            # removes the DMA load so PE-only time shows
                            q.dma_start(
                                ch[:, :ck, :, :32], src_ap[:, kk0 : kk0 + ck, :, :32]
                            )
                        else:
                            q.dma_start(
                                ch[:, :ck], src_ap[:, kk0 : kk0 + ck]
                            )
                        for j in range(ck):
                            kk = kk0 + j
                            if diag == "nomm" and kk > 0:
                                continue
                            stop_kk = 0 if diag == "nomm" else KK - 1
                            xt, xj = x_ts[kk]
                            for db in range(DBLK):
                                for mc in range(MC):
                                    nc.tensor.matmul(
                                        accs[db * MC + mc][:],
                                        xt[:, xj, :, db * P : (db + 1) * P],
                                        ch[:, j, :, mc * 512 : (mc + 1) * 512],
                                        start=(kk == 0),
                                        stop=(kk == stop_kk),
                                        perf_mode=DR,
                                    )
                    if not two_q:
                        _emit_v2_out(nc, o_pool, all_accs[mat], c_t, out_ap, mat, ODT)
                if two_q:
                    # copies/stores emitted after both chunk streams so the
                    # scalar ring's chunk dma_starts are never queued behind
                    # mat0-dependent work (dispatch is issue-and-go)
                    for mat in (0, 1):
                        _emit_v2_out(nc, o_pool, all_accs[mat], c_t, out_ap, mat, ODT)

    nc.compile()
    return nc


def _build_fp8dr(repeat=1):
    import concourse.bacc as bacc
    import concourse.tile as tile
    import concourse.mybir as mybir

    F32 = mybir.dt.float32
    FP8 = mybir.dt.float8e4
    DR = mybir.MatmulPerfMode.DoubleRow

    nc = bacc.Bacc(
        "TRN2",
        target_bir_lowering=False,
        debug=False,
        enable_asserts=False,
        num_devices=CORES,
    )
    at_ap = nc.dram_tensor("at", [N, M_LOC], FP8, kind="ExternalInput").ap()
    a2t_ap = nc.dram_tensor("a2t", [N, M_LOC], FP8, kind="ExternalInput").ap()
    x_ap = nc.dram_tensor("x", [N, D], FP8, kind="ExternalInput").ap()
    c_ap = nc.dram_tensor("c", [P, D], F32, kind="ExternalInput").ap()
    out_ap = nc.dram_tensor("out", [M_LOC, 2 * D], F32, kind="ExternalOutput").ap()

    XG = TUNE["xg"]
    with tile.TileContext(nc) as tc:
        with (
            tc.tile_pool(name="xp", bufs=1) as x_pool,
            tc.tile_pool(name="cp", bufs=1) as c_pool,
            tc.tile_pool(name="ap", bufs=TUNE["a_bufs"]) as a_pool,
            tc.tile_pool(name="op", bufs=MB) as o_pool,
            tc.tile_pool(name="pacc", bufs=MB, space="PSUM") as acc_pool,
        ):
            c_t = c_pool.tile([P, D], F32)
            nc.sync.dma_start(c_t[:], c_ap[:])

            x_t = x_pool.tile([P, KK, 2, D], FP8)
            x_re = x_ap.rearrange("(kk i p) d -> p kk i d", p=P, i=2)
            for g in range(KK // XG):
                nc.sync.dma_start(
                    x_t[:, g * XG : (g + 1) * XG], x_re[:, g * XG : (g + 1) * XG]
                )

            out_ts = [
                o_pool.tile([P, 2 * D], F32, tag="outt", name=f"outt{i}")
                for i in range(MB)
            ]
            for _rep in range(repeat):
                for mat, src_ap in ((0, at_ap), (1, a2t_ap)):
                    accs = [
                        acc_pool.tile([P, D], F32, tag="acc", name=f"acc{i}")
                        for i in range(MB)
                    ]
                    src_re = src_ap.rearrange("(kk i p) m -> p kk i m", p=P, i=2)
                    for kk in range(KK):
                        ch = a_pool.tile([P, 2, M_LOC], FP8, tag="achunk")
                        nc.sync.dma_start(ch[:], src_re[:, kk])
                        for mb in range(MB):
                            nc.tensor.matmul(
                                accs[mb][:],
                                ch[:, :, mb * P : (mb + 1) * P],
                                x_t[:, kk],
                                start=(kk == 0),
                                stop=(kk == KK - 1),
                                perf_mode=DR,
                            )
                    for mb in range(MB):
                        nc.vector.tensor_add(
                            out_ts[mb][:, mat * D : (mat + 1) * D],
                            accs[mb][:],
                            c_t[:],
                        )
                for mb in range(MB):
                    nc.sync.dma_start(out_ap[mb * P : (mb + 1) * P, :], out_ts[mb][:])

    nc.compile()
    return nc


def _build_bf16t(repeat=1):
    """Inputs pre-cast to bf16 on host (halves adjacency HBM traffic).
    A column-stripes [1024, 128] are loaded via the HW xbar DMA-transpose
    directly into matmul-ready [128k, 1024m] layout — no PE transposes, no
    PSUM round-trip. 8 PSUM banks hold one accumulator per output row-block."""
    import concourse.bacc as bacc
    import concourse.tile as tile
    import concourse.mybir as mybir

    F32 = mybir.dt.float32
    BF16 = mybir.dt.bfloat16

    nc = bacc.Bacc(
        "TRN2",
        target_bir_lowering=False,
        debug=False,
        enable_asserts=False,
        num_devices=CORES,
    )
    a_ap = nc.dram_tensor("a", [M_LOC, N], BF16, kind="ExternalInput").ap()
    a2_ap = nc.dram_tensor("a2", [M_LOC, N], BF16, kind="ExternalInput").ap()
    x_ap = nc.dram_tensor("x", [N, D], BF16, kind="ExternalInput").ap()
    out_ap = nc.dram_tensor("out", [M_LOC, 2 * D], F32, kind="ExternalOutput").ap()

    with tile.TileContext(nc) as tc:
        with (
            tc.tile_pool(name="xp", bufs=1) as x_pool,
            tc.tile_pool(name="stp", bufs=6) as st_pool,
            tc.tile_pool(name="op", bufs=MB) as o_pool,
            tc.tile_pool(name="pacc", bufs=MB, space="PSUM") as acc_pool,
        ):
            x_t = x_pool.tile([P, KB, D], BF16)
            x_re = x_ap.rearrange("(j p) d -> p j d", p=P)
            for g in range(N_GRP):
                nc.sync.dma_start(
                    x_t[:, g * GRP : (g + 1) * GRP, :],
                    x_re[:, g * GRP : (g + 1) * GRP, :],
                )

            out_ts = [
                o_pool.tile([P, 2 * D], F32, tag="outt", name=f"outt{i}")
                for i in range(MB)
            ]
            for _rep in range(repeat):
                for mat, src_ap in ((0, a_ap), (1, a2_ap)):
                    accs = [
                        acc_pool.tile([P, D], F32, tag="acc", name=f"acc{i}")
                        for i in range(MB)
                    ]
                    for k in range(KB):
                        st = st_pool.tile([P, M_LOC], BF16, tag="stripe")
                        nc.sync.dma_start_transpose(
                            st[:], src_ap[:, k * P : (k + 1) * P]
                        )
                        for mb in range(MB):
                            nc.tensor.matmul(
                                accs[mb][:],
                                st[:, mb * P : (mb + 1) * P],
                                x_t[:, k, :],
                                start=(k == 0),
                                stop=(k == KB - 1),
                            )
                    for mb in range(MB):
                        if mb % 2 == 0:
                            nc.vector.tensor_copy(
                                out_ts[mb][:, mat * D : (mat + 1) * D], accs[mb][:]
                            )
                        else:
                            nc.scalar.copy(
                                out_ts[mb][:, mat * D : (mat + 1) * D], accs[mb][:]
                            )
                for mb in range(MB):
                    nc.sync.dma_start(out_ap[mb * P : (mb + 1) * P, :], out_ts[mb][:])

    nc.compile()
    return nc


def _build_fp8drc(repeat=1):
    """Like fp8drb but each stationary x(kk, db) load feeds BOTH matrices'
    moving streams (4x 512-pair streams per LDWEIGHTS, 64 loads total).
    All 8 PSUM banks hold the two matrices' accumulators simultaneously."""
    import concourse.bacc as bacc
    import concourse.tile as tile
    import concourse.mybir as mybir

    F32 = mybir.dt.float32
    FP8 = mybir.dt.float8e4
    ODT = mybir.dt.bfloat16 if TUNE["out_bf16"] else F32
    DR = mybir.MatmulPerfMode.DoubleRow

    nc = bacc.Bacc(
        "TRN2",
        target_bir_lowering=False,
        debug=False,
        enable_asserts=False,
        num_devices=CORES,
    )
    at_ap = nc.dram_tensor("at", [N, M_LOC], FP8, kind="ExternalInput").ap()
    a2t_ap = nc.dram_tensor("a2t", [N, M_LOC], FP8, kind="ExternalInput").ap()
    x_ap = nc.dram_tensor("x", [N, D], FP8, kind="ExternalInput").ap()
    c_ap = nc.dram_tensor("c", [P, 2], F32, kind="ExternalInput").ap()
    out_ap = nc.dram_tensor("out", [2 * D, M_LOC], ODT, kind="ExternalOutput").ap()

    XG = TUNE["xg"]
    DBLK = D // P  # 2 stationary d-blocks
    MC = M_LOC // 512  # 2 moving chunks of 512 columns
    with tile.TileContext(nc) as tc:
        with (
            tc.tile_pool(name="xp", bufs=1) as x_pool,
            tc.tile_pool(name="cp", bufs=1) as c_pool,
            tc.tile_pool(name="ap", bufs=TUNE["a_bufs"]) as a_pool,
            tc.tile_pool(name="op", bufs=2 * DBLK * MC) as o_pool,
            tc.tile_pool(name="pacc", bufs=2 * DBLK * MC, space="PSUM") as acc_pool,
        ):
            c_t = c_pool.tile([P, 2], F32)
            nc.gpsimd.dma_start(c_t[:], c_ap[:])

            x_t = x_pool.tile([P, KK, 2, D], FP8)
            x_re = x_ap.rearrange("(kk i p) d -> p kk i d", p=P, i=2)

            if TUNE["il"]:
                # host stored rows as (kk, p, i): per partition the k-pair is
                # one contiguous 2048 B run
                at_re = at_ap.rearrange("(kk p i) m -> p kk i m", p=P, i=2)
                a2t_re = a2t_ap.rearrange("(kk p i) m -> p kk i m", p=P, i=2)
            else:
                at_re = at_ap.rearrange("(kk i p) m -> p kk i m", p=P, i=2)
                a2t_re = a2t_ap.rearrange("(kk i p) m -> p kk i m", p=P, i=2)

            hwloop = repeat > 1 and TUNE["hwloop"]
            if hwloop:
                # hardware loop: x fully loaded upfront, body emitted once
                for g in range(KK // XG):
                    nc.gpsimd.dma_start(
                        x_t[:, g * XG : (g + 1) * XG], x_re[:, g * XG : (g + 1) * XG]
                    )
                rep_iter = [0]
                loop_cm = tc.For_i(0, repeat)
                loop_cm.__enter__()
            else:
                rep_iter = list(range(repeat))
            for _rep in rep_iter:
                # acc index: mat*4 + db*2 + mc
                accs = [
                    acc_pool.tile([P, 512], F32, tag="acc", name=f"acc{i}")
                    for i in range(2 * DBLK * MC)
                ]
                diag = TUNE["diag"]
                for kk in range(KK):
                    if not hwloop and _rep == 0 and kk % XG == 0:
                        nc.gpsimd.dma_start(
                            x_t[:, kk : kk + XG], x_re[:, kk : kk + XG]
                        )
                    ch = a_pool.tile([P, 2, M_LOC], FP8, tag="achunk")
                    ch2 = a_pool.tile([P, 2, M_LOC], FP8, tag="achunk")
                    if diag == "nodma":
                        # 1/32 of the bytes: keeps the dep structure, removes
                        # the DMA load so PE-only time is visible
                        nc.sync.dma_start(ch[:, :, :32], at_re[:, kk, :, :32])
                        nc.scalar.dma_start(ch2[:, :, :32], a2t_re[:, kk, :, :32])
                    else:
                        # two HWDGE queues in parallel: at on sync, a2t on
                        # scalar (each ~8 MB/rep)
                        nc.sync.dma_start(ch[:], at_re[:, kk])
                        nc.scalar.dma_start(ch2[:], a2t_re[:, kk])
                    if diag == "nomm" and kk > 0:
                        continue
                    stop_kk = 0 if diag == "nomm" else KK - 1
                    for db in range(DBLK):
                        for mat, c_ch in ((0, ch), (1, ch2)):
                            for mc in range(MC):
                                nc.tensor.matmul(
                                    accs[mat * 4 + db * MC + mc][:],
                                    x_t[:, kk, :, db * P : (db + 1) * P],
                                    c_ch[:, :, mc * 512 : (mc + 1) * 512],
                                    start=(kk == 0),
                                    stop=(kk == stop_kk),
                                    perf_mode=DR,
                                )
                for mat in range(2):
                    for db in range(DBLK):
                        for mc in range(MC):
                            ot = o_pool.tile([P, 512], ODT, tag="outt")
                            if (db * MC + mc) % 2 == 0:
                                nc.vector.tensor_scalar_add(
                                    ot[:], accs[mat * 4 + db * MC + mc][:],
                                    c_t[:, db : db + 1],
                                )
                            else:
                                nc.scalar.add(
                                    ot[:], accs[mat * 4 + db * MC + mc][:],
                                    c_t[:, db : db + 1],
                                )
                            nc.scalar.dma_start(
                                out_ap[
                                    mat * D + db * P : mat * D + (db + 1) * P,
                                    mc * 512 : (mc + 1) * 512,
                                ],
                                ot[:],
                            )
            if hwloop:
                loop_cm.__exit__(None, None, None)

    nc.compile()
    return nc


def _build(mode, repeat=1):
    if mode == "fp8v2":
        return _build_fp8v2(repeat)
    if mode == "fp8drc":
        return _build_fp8drc(repeat)
    if mode == "fp8drb":
        return _build_fp8drb(repeat)
    if mode == "fp8dr":
        return _build_fp8dr(repeat)
    if mode == "bf16t":
        return _build_bf16t(repeat)
    raise ValueError(f"unknown mode {mode}")


def _get_nc(mode, repeat=1):
    key = (mode, repeat, tuple(sorted(TUNE.items())))
    if key not in _cache:
        _cache[key] = _build(mode, repeat)
    return _cache[key]


def make_in_maps(x, adj_t, adj_t2, mode=MODE):
    import ml_dtypes

    x = np.ascontiguousarray(np.asarray(x, dtype=np.float32))
    adj_t = np.asarray(adj_t, dtype=np.float32)
    adj_t2 = np.asarray(adj_t2, dtype=np.float32)
    if mode == "fp8v2":
        e4 = ml_dtypes.float8_e4m3
        # x_v2[p, kk, i, d] = xq[kk*256 + i*128 + p, d]
        xq = np.ascontiguousarray(
            x.astype(e4).reshape(KK, 2, P, D).transpose(2, 0, 1, 3)
        )
        c_row = (0.5 * x.sum(0, dtype=np.float64)).astype(np.float32)
        c = np.ascontiguousarray(c_row.reshape(2, P).T)  # [P, 2] d-blocks

        def prep(adj, sl):
            # A_v2[p, kk, i, m] = fp8(adj[sl][m, kk*256+i*128+p] - 0.5)
            a = (adj[sl] - 0.5).astype(e4)  # [M_LOC, N]
            return np.ascontiguousarray(
                a.reshape(M_LOC, KK, 2, P).transpose(3, 1, 2, 0)
            )

        return [
            {
                "at": prep(adj_t, slice(cid * M_LOC, (cid + 1) * M_LOC)),
                "a2t": prep(adj_t2, slice(cid * M_LOC, (cid + 1) * M_LOC)),
                "x": xq,
                "c": c,
            }
            for cid in range(CORES)
        ]
    if mode in ("fp8dr", "fp8drb", "fp8drc"):
        e4 = ml_dtypes.float8_e4m3
        xq = x.astype(e4)
        c_row = (0.5 * x.sum(0, dtype=np.float64)).astype(np.float32)
        if mode in ("fp8drb", "fp8drc"):
            c = np.ascontiguousarray(c_row.reshape(2, P).T)  # [P, 2] d-blocks
        else:
            c = np.ascontiguousarray(np.broadcast_to(c_row, (P, D)))
        def prep(adj, sl):
            at = np.ascontiguousarray((adj[sl] - 0.5).astype(e4).T)  # [N, M_LOC]
            if mode == "fp8drc" and TUNE["il"]:
                # reorder k-rows (kk, i, p) -> (kk, p, i) so each partition's
                # DoubleRow pair is contiguous in DRAM
                at = np.ascontiguousarray(
                    at.reshape(KK, 2, P, M_LOC).swapaxes(1, 2).reshape(N, M_LOC)
                )
            return at

        maps = []
        for cid in range(CORES):
            sl = slice(cid * M_LOC, (cid + 1) * M_LOC)
            maps.append(
                {
                    "at": prep(adj_t, sl),
                    "a2t": prep(adj_t2, sl),
                    "x": xq,
                    "c": c,
                }
            )
        return maps
    bf = ml_dtypes.bfloat16
    xb = x.astype(bf)
    ab = adj_t.astype(bf)
    a2b = adj_t2.astype(bf)
    return [
        {
            "a": ab[c * M_LOC : (c + 1) * M_LOC],
            "a2": a2b[c * M_LOC : (c + 1) * M_LOC],
            "x": xb,
        }
        for c in range(CORES)
    ]


def gather_out(results, mode=MODE):
    if mode in ("fp8drb", "fp8drc", "fp8v2"):
        return np.concatenate(
            [np.ascontiguousarray(r["out"].T).astype(np.float32) for r in results],
            axis=0,
        )
    return np.concatenate([r["out"] for r in results], axis=0)


def kernel(x, adj_t, adj_t2):
    from concourse.bass_utils import run_bass_kernel_spmd

    nc = _get_nc(MODE)
    in_maps = make_in_maps(x, adj_t, adj_t2, MODE)
    res = run_bass_kernel_spmd(nc, in_maps, core_ids=list(range(CORES)))
    return gather_out(res.results, MODE)



# revision 34
# speedup vs baseline: 1.3693x; 1.3693x over previous
"""H2GCN neighborhood aggregation on 8 Trainium2 NeuronCores.

Computes concat([adj_t @ x, adj_t2 @ x], axis=1) for
adj_t/adj_t2: [8192, 8192] f32, x: [8192, 256] f32.

Sharding: row-shard adj_t/adj_t2 (1024 rows per core), replicate x,
each core produces its [1024, 512] slice of the output.

fp8v2 mode (default): adjacency is centered (a - 0.5) and quantized to
fp8 e4m3 on host, x quantized to e4m3, and the rank-1 term
0.5 * colsum(x) is carried exactly in f32 and added after accumulation.
Matmuls run in MatmulPerfMode.DoubleRow (256 contraction rows per
instruction, 2x PE throughput); measured ~153 TFLOP/s/core, ~98% of the
fp8 PE peak. Adjacency HBM traffic is halved vs bf16 so DMA (~41 us)
hides fully under the PE stream (~56 us). Measured rel err vs the f32
reference: 1.4e-2 (gate 2e-2).

Per-core dataflow (fp8v2):
  - host packs the adjacency slice p-major [128p, 32kk, 2, 1024m] e4m3
    (64 KB contiguous per partition) and x as [128p, 32kk, 2, 256d], so
    every DMA descriptor is a single >=2KB contiguous run.
  - adjacency streams in CKK-kk chunks on the sync HWDGE queue; x lands
    in staged per-group tiles (first tiny group leads the sync ring so
    the first matmul un-gates at ~2.5us; tile-granular dep tracking
    would otherwise gate every matmul on the last x DMA). chunk k's
    4*CKK DoubleRow matmuls accumulate out^T blocks [128d, 512m] in the
    8 PSUM banks (both matrices resident, so the PE never drains
    between matrices).
  - a few throwaway matmuls at t=0 lift the PE HAM clock gate (cold PE
    runs at 1.2 GHz for ~3.4 us) while the first DMAs are in flight.
  - mat0's first / mat1's last chunk are split into single-kk pieces to
    shorten the DMA-gated head and the accumulate->copy->store tail.
  - DVE/Act add the f32 colsum bias during the PSUM -> SBUF bf16 copy;
    stores are per-(mat, d-block) [128, 1024] bf16 rows, the very last
    on the otherwise-idle sync queue. Host un-transposes and upcasts.
"""

import numpy as np

N = 8192
D = 256
CORES = 8
P = 128
M_LOC = N // CORES  # 1024 rows of each adjacency matrix per core
MB = M_LOC // P  # 8 output row-blocks per core
KB = N // P  # 64 contraction blocks
KK = N // (2 * P)  # 32 k-pair blocks (DoubleRow consumes 256 rows/step)
GRP = 8  # k-blocks per transpose/copy group (one PSUM bank), bf16t mode
N_GRP = KB // GRP  # 8

MODE = "fp8v2"  # "fp8v2" / "fp8drb" / "fp8dr" (e4m3 DoubleRow) / "bf16t"

TUNE = dict(
    a_bufs=12,  # fp8 chunk slots in flight
    xg=4,  # kk-blocks per x-load DMA
    chunk_kk=1,  # kk-pairs per adjacency chunk DMA
    out_bf16=True,  # store out as bf16, host upcasts
    diag="none",  # "nodma" (tiny chunk DMAs) / "nomm" (single matmul per acc)
    hwloop=True,  # use tc.For_i for the repeat loop (timing NEFFs only)
    il=True,  # host pair-interleaves adjacency rows (2KB contiguous/partition)
    v2_ckk=4,  # kk-pairs per adjacency chunk DMA (v2)
    v2_xg=8,  # kk-pairs per x-load DMA (v2)
    v2_abufs=8,  # adjacency chunk slots in flight (v2)
    v2_warm=20,  # dummy N=128 matmuls at t=0 to lift the HAM clock gate
    # (sized so PE activity is continuous from ~0.3us until the first
    # DMA-gated real matmul at ~2.3us — an idle gap would let the HAM's
    # free-running idle window re-throttle the clock, while surplus
    # dummies would delay the first real matmul)
    v2_2q=False,  # alternate chunk DMAs across both HWDGE rings
)

_cache = {}


def _build_fp8drb(repeat=1):
    """x-stationary orientation: out^T[d, m] accumulates in PSUM while the
    adjacency chunk streams as the 1024-wide moving operand (512 k-pairs).
    Stationary x blocks are reused across 2 moving streams, so LD_WEIGHTS
    is amortized 4x vs the a-stationary orientation. The colsum correction
    is a per-partition bias add; host un-transposes the [512, 1024] out.
    Output is stored bf16 (host upcasts); x loads stream on the gpsimd
    DMA queue interleaved with the first matrix's chunks."""
    import concourse.bacc as bacc
    import concourse.tile as tile
    import concourse.mybir as mybir

    F32 = mybir.dt.float32
    FP8 = mybir.dt.float8e4
    ODT = mybir.dt.bfloat16 if TUNE["out_bf16"] else F32
    DR = mybir.MatmulPerfMode.DoubleRow

    nc = bacc.Bacc(
        "TRN2",
        target_bir_lowering=False,
        debug=False,
        enable_asserts=False,
        num_devices=CORES,
    )
    at_ap = nc.dram_tensor("at", [N, M_LOC], FP8, kind="ExternalInput").ap()
    a2t_ap = nc.dram_tensor("a2t", [N, M_LOC], FP8, kind="ExternalInput").ap()
    x_ap = nc.dram_tensor("x", [N, D], FP8, kind="ExternalInput").ap()
    c_ap = nc.dram_tensor("c", [P, 2], F32, kind="ExternalInput").ap()
    # out is transposed: [2D, M_LOC]; host transposes back (and upcasts)
    out_ap = nc.dram_tensor("out", [2 * D, M_LOC], ODT, kind="ExternalOutput").ap()

    XG = TUNE["xg"]
    CKK = TUNE["chunk_kk"]  # kk-pairs per adjacency chunk DMA
    DBLK = D // P  # 2 stationary d-blocks
    MC = M_LOC // 512  # 2 moving chunks of 512 columns
    with tile.TileContext(nc) as tc:
        with (
            tc.tile_pool(name="xp", bufs=1) as x_pool,
            tc.tile_pool(name="cp", bufs=1) as c_pool,
            tc.tile_pool(name="ap", bufs=TUNE["a_bufs"]) as a_pool,
            tc.tile_pool(name="op", bufs=2 * DBLK * MC) as o_pool,
            tc.tile_pool(name="pacc", bufs=2 * DBLK * MC, space="PSUM") as acc_pool,
        ):
            c_t = c_pool.tile([P, 2], F32)
            nc.gpsimd.dma_start(c_t[:], c_ap[:])

            x_t = x_pool.tile([P, KK, 2, D], FP8)
            x_re = x_ap.rearrange("(kk i p) d -> p kk i d", p=P, i=2)

            for _rep in range(repeat):
                for mat, src_ap in ((0, at_ap), (1, a2t_ap)):
                    accs = [
                        acc_pool.tile([P, 512], F32, tag="acc", name=f"acc{i}")
                        for i in range(DBLK * MC)
                    ]
                    src_re = src_ap.rearrange("(kk i p) m -> p kk i m", p=P, i=2)
                    for kk0 in range(0, KK, CKK):
                        if _rep == 0 and mat == 0 and kk0 % XG == 0:
                            # x chunk rides the gpsimd queue, just ahead of
                            # the adjacency chunks that need it
                            nc.gpsimd.dma_start(
                                x_t[:, kk0 : kk0 + XG], x_re[:, kk0 : kk0 + XG]
                            )
                        ch = a_pool.tile([P, CKK, 2, M_LOC], FP8, tag="achunk")
                        nc.sync.dma_start(ch[:], src_re[:, kk0 : kk0 + CKK])
                        for j in range(CKK):
                            kk = kk0 + j
                            for db in range(DBLK):
                                for mc in range(MC):
                                    nc.tensor.matmul(
                                        accs[db * MC + mc][:],
                                        x_t[:, kk, :, db * P : (db + 1) * P],
                                        ch[:, j, :, mc * 512 : (mc + 1) * 512],
                                        start=(kk == 0),
                                        stop=(kk == KK - 1),
                                        perf_mode=DR,
                                    )
                    for db in range(DBLK):
                        for mc in range(MC):
                            ot = o_pool.tile([P, 512], ODT, tag="outt")
                            if (db * MC + mc) % 2 == 0:
                                nc.vector.tensor_scalar_add(
                                    ot[:], accs[db * MC + mc][:], c_t[:, db : db + 1]
                                )
                            else:
                                nc.scalar.add(
                                    ot[:], accs[db * MC + mc][:], c_t[:, db : db + 1]
                                )
                            nc.scalar.dma_start(
                                out_ap[
                                    mat * D + db * P : mat * D + (db + 1) * P,
                                    mc * 512 : (mc + 1) * 512,
                                ],
                                ot[:],
                            )

    nc.compile()
    return nc


def _emit_v2_out(nc, o_pool, accs, c_t, out_ap, mat, ODT):
    """PSUM -> SBUF bf16 copies (+colsum bias) and per-d-block out stores."""
    import concourse.mybir as mybir  # noqa: F401

    P_, D_, MC_ = P, D, 2
    for db in range(2):
        ot = o_pool.tile([P_, 2, 512], ODT, tag="outt")
        for mc in range(MC_):
            # one full-width copy per engine: splitting these across both
            # engines halves the per-acc drain but serializes 8 half-copies
            # plus store descriptor-gen on Act and sims 1.5us WORSE overall
            if mc == 0:
                nc.vector.tensor_scalar_add(
                    ot[:, mc], accs[db * MC_ + mc][:], c_t[:, db : db + 1]
                )
            else:
                nc.scalar.add(
                    ot[:, mc], accs[db * MC_ + mc][:], c_t[:, db : db + 1]
                )
        # mat0 stores must stay off sync so mat1's chunk stream is never
        # queued behind them (they're emitted before mat1's chunk loop);
        # mat1's stores ride the by-then-idle sync ring so their HWDGE
        # gens don't serialize with the Act-side tail copies
        rows = slice(mat * D_ + db * P_, mat * D_ + (db + 1) * P_)
        if mat == 1 and db == 1:
            # final store: split the two m-halves across both HWDGE rings
            # so the tail's last transfer is half-length and issues in
            # parallel as each copy lands
            nc.sync.dma_start(out_ap[rows, 0:512], ot[:, 0])
            nc.scalar.dma_start(out_ap[rows, 512:1024], ot[:, 1])
        elif mat == 1:
            nc.sync.dma_start(
                out_ap[rows, :], ot[:].rearrange("p mc m -> p (mc m)")
            )
        else:
            nc.scalar.dma_start(
                out_ap[rows, :], ot[:].rearrange("p mc m -> p (mc m)")
            )


def _build_fp8v2(repeat=1):
    """x-stationary orientation like fp8drb, with p-major DRAM layouts so
    every DMA descriptor is a single >=2KB contiguous run per partition:
      at/a2t: [P, KK, 2, M_LOC] fp8 (64 KB contiguous per partition),
      x:      [P, KK, 2, D]     fp8 (16 KB per partition).
    Queue plan (avoids prefetch-blocking): all adjacency chunks stream on
    sync (SP); x + c ride vector (DVE); PSUM->SBUF copies alternate
    vector/scalar; out stores are per-(mat, db) row-blocks [P, 2D] bf16 on
    scalar. Both matrices' accumulators live in the 8 PSUM banks so mat1
    matmuls start while mat0 drains."""
    import concourse.bacc as bacc
    import concourse.tile as tile
    import concourse.mybir as mybir

    F32 = mybir.dt.float32
    FP8 = mybir.dt.float8e4
    ODT = mybir.dt.bfloat16 if TUNE["out_bf16"] else F32
    DR = mybir.MatmulPerfMode.DoubleRow

    nc = bacc.Bacc(
        "TRN2",
        target_bir_lowering=False,
        debug=False,
        enable_asserts=False,
        num_devices=CORES,
    )
    at_ap = nc.dram_tensor("at", [P, KK, 2, M_LOC], FP8, kind="ExternalInput").ap()
    a2t_ap = nc.dram_tensor("a2t", [P, KK, 2, M_LOC], FP8, kind="ExternalInput").ap()
    x_ap = nc.dram_tensor("x", [P, KK, 2, D], FP8, kind="ExternalInput").ap()
    c_ap = nc.dram_tensor("c", [P, 2], F32, kind="ExternalInput").ap()
    # out is transposed: [2D, M_LOC]; host transposes back (and upcasts)
    out_ap = nc.dram_tensor("out", [2 * D, M_LOC], ODT, kind="ExternalOutput").ap()

    CKK = TUNE["v2_ckk"]  # kk-pairs per adjacency chunk DMA
    XG = TUNE["v2_xg"]  # kk-pairs per x DMA
    DBLK = D // P  # 2 stationary d-blocks
    MC = M_LOC // 512  # 2 moving chunks of 512 columns
    with tile.TileContext(nc) as tc:
        with (
            tc.tile_pool(name="xp", bufs=4) as x_pool,
            tc.tile_pool(name="cp", bufs=1) as c_pool,
            tc.tile_pool(name="hp", bufs=2) as h_pool,
            tc.tile_pool(name="ap", bufs=TUNE["v2_abufs"]) as a_pool,
            tc.tile_pool(name="op", bufs=2 * DBLK) as o_pool,
            tc.tile_pool(name="pacc", bufs=2 * DBLK * MC, space="PSUM") as acc_pool,
        ):
            if TUNE["v2_warm"]:
                # lift the PE HAM clock gate during the DMA ramp: a zeroed
                # fp8 tile feeds throwaway N=128 matmuls into acc bank 0;
                # the real accumulation's start=True reset makes them inert.
                # memset on vector so the gpsimd x-load queue isn't delayed.
                warm_t = c_pool.tile([P, 2, P], FP8)
                nc.vector.memset(warm_t[:], 0.0)

            # x arrives in staged groups, each its OWN tile: tile-granular
            # dependency tracking would otherwise gate every matmul on the
            # LAST x DMA (~+3.5us on the one-shot critical path). The first
            # tiny group is the very first transfer on the sync HWDGE ring
            # (~0.6us first-byte) so the first matmul un-gates immediately;
            # later groups land well before the PE stream reaches them.
            xgroups = [(0, 2), (2, 8), (10, 11), (21, KK - 21)]
            x_ts = {}
            for gi, (g0, gn) in enumerate(xgroups):
                xt = x_pool.tile([P, gn, 2, D], FP8, name=f"xg{gi}")
                q = nc.sync if gi == 0 else nc.gpsimd
                q.dma_start(xt[:], x_ap[:, g0 : g0 + gn])
                for kk in range(g0, g0 + gn):
                    x_ts[kk] = (xt, kk - g0)
            c_t = c_pool.tile([P, 2], F32)
            nc.gpsimd.dma_start(c_t[:], c_ap[:])

            # chunk plans: CKK-sized chunks; mat0's first block split fine so
            # the first matmul isn't gated on a large DMA, mat1's last block
            # split fine so the tail chain is short
            def mk_plan(head_split, tail_split):
                plan, kk0 = [], 0
                while kk0 < KK:
                    ck = min(CKK, KK - kk0)
                    fine = (head_split and kk0 == 0) or (
                        tail_split and kk0 + ck >= KK
                    )
                    if fine and ck > 1:
                        plan.extend((kk0 + j, 1) for j in range(ck))
                    else:
                        plan.append((kk0, ck))
                    kk0 += ck
                return plan

            if TUNE["v2_warm"]:
                warm_acc = acc_pool.tile([P, 512], F32, tag="acc", name="warm")
                for w in range(TUNE["v2_warm"]):
                    nc.tensor.matmul(
                        warm_acc[:, :128],
                        warm_t[:],
                        warm_t[:],
                        start=True,
                        stop=True,
                        perf_mode=DR,
                        skip_group_check=True,
                    )

            diag = TUNE["diag"]
            two_q = TUNE["v2_2q"]
            for _rep in range(repeat):
                all_accs = {}
                qi = 0
                for mat, src_ap in ((0, at_ap), (1, a2t_ap)):
                    plan = mk_plan(head_split=(_rep == 0 and mat == 0), tail_split=(mat == 1))
                    accs = [
                        acc_pool.tile([P, 512], F32, tag="acc", name=f"acc{mat}_{i}")
                        for i in range(DBLK * MC)
                    ]
                    all_accs[mat] = accs
                    head0 = _rep == 0 and mat == 0
                    for kk0, ck in plan:
                        if head0 and kk0 == 0 and diag == "none":
                            # split chunk0 into per-mc-half tiles on BOTH
                            # HWDGE rings: the mc0 half (364ns, leading the
                            # otherwise-empty scalar ring) un-gates the
                            # first matmuls while the mc1 half + xg0 land
                            # on sync. Separate tiles, not slices — dep
                            # tracking is tile-granular.
                            cha = h_pool.tile([P, 1, 2, 512], FP8, tag="ch0")
                            chb = h_pool.tile([P, 1, 2, 512], FP8, tag="ch0")
                            nc.scalar.dma_start(cha[:], src_ap[:, 0:1, :, 0:512])
                            nc.sync.dma_start(chb[:], src_ap[:, 0:1, :, 512:1024])
                            xt, xj = x_ts[0]
                            for db in range(DBLK):
                                for mc, c_ch in ((0, cha), (1, chb)):
                                    nc.tensor.matmul(
                                        accs[db * MC + mc][:],
                                        xt[:, xj, :, db * P : (db + 1) * P],
                                        c_ch[:, 0],
                                        start=True,
                                        stop=False,
                                        perf_mode=DR,
                                    )
                            continue
                        ch = a_pool.tile([P, CKK, 2, M_LOC], FP8, tag="achunk")
                        q = nc.scalar if (two_q and qi % 2) else nc.sync
                        qi += 1
                        if diag == "nodma":
                            # 1/32 of the bytes: keeps the dep structure,
                            # removes the DMA load so PE-only time shows
                            q.dma_start(
                                ch[:, :ck, :, :32], src_ap[:, kk0 : kk0 + ck, :, :32]
                            )
                        else:
                            q.dma_start(
                                ch[:, :ck], src_ap[:, kk0 : kk0 + ck]
                            )
                        for j in range(ck):
                            kk = kk0 + j
                            if diag == "nomm" and kk > 0:
                                continue
                            stop_kk = 0 if diag == "nomm" else KK - 1
                            xt, xj = x_ts[kk]
                            for db in range(DBLK):
                                for mc in range(MC):
                                    nc.tensor.matmul(
                                        accs[db * MC + mc][:],
                                        xt[:, xj, :, db * P : (db + 1) * P],
                                        ch[:, j, :, mc * 512 : (mc + 1) * 512],
                                        start=(kk == 0),
                                        stop=(kk == stop_kk),
                                        perf_mode=DR,
                                    )
                    if not two_q:
                        _emit_v2_out(nc, o_pool, all_accs[mat], c_t, out_ap, mat, ODT)
                if two_q:
                    # copies/stores emitted after both chunk streams so the
                    # scalar ring's chunk dma_starts are never queued behind
                    # mat0-dependent work (dispatch is issue-and-go)
                    for mat in (0, 1):
                        _emit_v2_out(nc, o_pool, all_accs[mat], c_t, out_ap, mat, ODT)

    nc.compile()
    return nc


def _build_fp8dr(repeat=1):
    import concourse.bacc as bacc
    import concourse.tile as tile
    import concourse.mybir as mybir

    F32 = mybir.dt.float32
    FP8 = mybir.dt.float8e4
    DR = mybir.MatmulPerfMode.DoubleRow

    nc = bacc.Bacc(
        "TRN2",
        target_bir_lowering=False,
        debug=False,
        enable_asserts=False,
        num_devices=CORES,
    )
    at_ap = nc.dram_tensor("at", [N, M_LOC], FP8, kind="ExternalInput").ap()
    a2t_ap = nc.dram_tensor("a2t", [N, M_LOC], FP8, kind="ExternalInput").ap()
    x_ap = nc.dram_tensor("x", [N, D], FP8, kind="ExternalInput").ap()
    c_ap = nc.dram_tensor("c", [P, D], F32, kind="ExternalInput").ap()
    out_ap = nc.dram_tensor("out", [M_LOC, 2 * D], F32, kind="ExternalOutput").ap()

    XG = TUNE["xg"]
    with tile.TileContext(nc) as tc:
        with (
            tc.tile_pool(name="xp", bufs=1) as x_pool,
            tc.tile_pool(name="cp", bufs=1) as c_pool,
            tc.tile_pool(name="ap", bufs=TUNE["a_bufs"]) as a_pool,
            tc.tile_pool(name="op", bufs=MB) as o_pool,
            tc.tile_pool(name="pacc", bufs=MB, space="PSUM") as acc_pool,
        ):
            c_t = c_pool.tile([P, D], F32)
            nc.sync.dma_start(c_t[:], c_ap[:])

            x_t = x_pool.tile([P, KK, 2, D], FP8)
            x_re = x_ap.rearrange("(kk i p) d -> p kk i d", p=P, i=2)
            for g in range(KK // XG):
                nc.sync.dma_start(
                    x_t[:, g * XG : (g + 1) * XG], x_re[:, g * XG : (g + 1) * XG]
                )

            out_ts = [
                o_pool.tile([P, 2 * D], F32, tag="outt", name=f"outt{i}")
                for i in range(MB)
            ]
            for _rep in range(repeat):
                for mat, src_ap in ((0, at_ap), (1, a2t_ap)):
                    accs = [
                        acc_pool.tile([P, D], F32, tag="acc", name=f"acc{i}")
                        for i in range(MB)
                    ]
                    src_re = src_ap.rearrange("(kk i p) m -> p kk i m", p=P, i=2)
                    for kk in range(KK):
                        ch = a_pool.tile([P, 2, M_LOC], FP8, tag="achunk")
                        nc.sync.dma_start(ch[:], src_re[:, kk])
                        for mb in range(MB):
                            nc.tensor.matmul(
                                accs[mb][:],
                                ch[:, :, mb * P : (mb + 1) * P],
                                x_t[:, kk],
                                start=(kk == 0),
                                stop=(kk == KK - 1),
                                perf_mode=DR,
                            )
                    for mb in range(MB):
                        nc.vector.tensor_add(
                            out_ts[mb][:, mat * D : (mat + 1) * D],
                            accs[mb][:],
                            c_t[:],
                        )
                for mb in range(MB):
                    nc.sync.dma_start(out_ap[mb * P : (mb + 1) * P, :], out_ts[mb][:])

    nc.compile()
    return nc


def _build_bf16t(repeat=1):
    """Inputs pre-cast to bf16 on host (halves adjacency HBM traffic).
    A column-stripes [1024, 128] are loaded via the HW xbar DMA-transpose
    directly into matmul-ready [128k, 1024m] layout — no PE transposes, no
    PSUM round-trip. 8 PSUM banks hold one accumulator per output row-block."""
    import concourse.bacc as bacc
    import concourse.tile as tile
    import concourse.mybir as mybir

    F32 = mybir.dt.float32
    BF16 = mybir.dt.bfloat16

    nc = bacc.Bacc(
        "TRN2",
        target_bir_lowering=False,
        debug=False,
        enable_asserts=False,
        num_devices=CORES,
    )
    a_ap = nc.dram_tensor("a", [M_LOC, N], BF16, kind="ExternalInput").ap()
    a2_ap = nc.dram_tensor("a2", [M_LOC, N], BF16, kind="ExternalInput").ap()
    x_ap = nc.dram_tensor("x", [N, D], BF16, kind="ExternalInput").ap()
    out_ap = nc.dram_tensor("out", [M_LOC, 2 * D], F32, kind="ExternalOutput").ap()

    with tile.TileContext(nc) as tc:
        with (
            tc.tile_pool(name="xp", bufs=1) as x_pool,
            tc.tile_pool(name="stp", bufs=6) as st_pool,
            tc.tile_pool(name="op", bufs=MB) as o_pool,
            tc.tile_pool(name="pacc", bufs=MB, space="PSUM") as acc_pool,
        ):
            x_t = x_pool.tile([P, KB, D], BF16)
            x_re = x_ap.rearrange("(j p) d -> p j d", p=P)
            for g in range(N_GRP):
                nc.sync.dma_start(
                    x_t[:, g * GRP : (g + 1) * GRP, :],
                    x_re[:, g * GRP : (g + 1) * GRP, :],
                )

            out_ts = [
                o_pool.tile([P, 2 * D], F32, tag="outt", name=f"outt{i}")
                for i in range(MB)
            ]
            for _rep in range(repeat):
                for mat, src_ap in ((0, a_ap), (1, a2_ap)):
                    accs = [
                        acc_pool.tile([P, D], F32, tag="acc", name=f"acc{i}")
                        for i in range(MB)
                    ]
                    for k in range(KB):
                        st = st_pool.tile([P, M_LOC], BF16, tag="stripe")
                        nc.sync.dma_start_transpose(
                            st[:], src_ap[:, k * P : (k + 1) * P]
                        )
                        for mb in range(MB):
                            nc.tensor.matmul(
                                accs[mb][:],
                                st[:, mb * P : (mb + 1) * P],
                                x_t[:, k, :],
                                start=(k == 0),
                                stop=(k == KB - 1),
                            )
                    for mb in range(MB):
                        if mb % 2 == 0:
                            nc.vector.tensor_copy(
                                out_ts[mb][:, mat * D : (mat + 1) * D], accs[mb][:]
                            )
                        else:
                            nc.scalar.copy(
                                out_ts[mb][:, mat * D : (mat + 1) * D], accs[mb][:]
                            )
                for mb in range(MB):
                    nc.sync.dma_start(out_ap[mb * P : (mb + 1) * P, :], out_ts[mb][:])

    nc.compile()
    return nc


def _build_fp8drc(repeat=1):
    """Like fp8drb but each stationary x(kk, db) load feeds BOTH matrices'
    moving streams (4x 512-pair streams per LDWEIGHTS, 64 loads total).
    All 8 PSUM banks hold the two matrices' accumulators simultaneously."""
    import concourse.bacc as bacc
    import concourse.tile as tile
    import concourse.mybir as mybir

    F32 = mybir.dt.float32
    FP8 = mybir.dt.float8e4
    ODT = mybir.dt.bfloat16 if TUNE["out_bf16"] else F32
    DR = mybir.MatmulPerfMode.DoubleRow

    nc = bacc.Bacc(
        "TRN2",
        target_bir_lowering=False,
        debug=False,
        enable_asserts=False,
        num_devices=CORES,
    )
    at_ap = nc.dram_tensor("at", [N, M_LOC], FP8, kind="ExternalInput").ap()
    a2t_ap = nc.dram_tensor("a2t", [N, M_LOC], FP8, kind="ExternalInput").ap()
    x_ap = nc.dram_tensor("x", [N, D], FP8, kind="ExternalInput").ap()
    c_ap = nc.dram_tensor("c", [P, 2], F32, kind="ExternalInput").ap()
    out_ap = nc.dram_tensor("out", [2 * D, M_LOC], ODT, kind="ExternalOutput").ap()

    XG = TUNE["xg"]
    DBLK = D // P  # 2 stationary d-blocks
    MC = M_LOC // 512  # 2 moving chunks of 512 columns
    with tile.TileContext(nc) as tc:
        with (
            tc.tile_pool(name="xp", bufs=1) as x_pool,
            tc.tile_pool(name="cp", bufs=1) as c_pool,
            tc.tile_pool(name="ap", bufs=TUNE["a_bufs"]) as a_pool,
            tc.tile_pool(name="op", bufs=2 * DBLK * MC) as o_pool,
            tc.tile_pool(name="pacc", bufs=2 * DBLK * MC, space="PSUM") as acc_pool,
        ):
            c_t = c_pool.tile([P, 2], F32)
            nc.gpsimd.dma_start(c_t[:], c_ap[:])

            x_t = x_pool.tile([P, KK, 2, D], FP8)
            x_re = x_ap.rearrange("(kk i p) d -> p kk i d", p=P, i=2)

            if TUNE["il"]:
                # host stored rows as (kk, p, i): per partition the k-pair is
                # one contiguous 2048 B run
                at_re = at_ap.rearrange("(kk p i) m -> p kk i m", p=P, i=2)
                a2t_re = a2t_ap.rearrange("(kk p i) m -> p kk i m", p=P, i=2)
            else:
                at_re = at_ap.rearrange("(kk i p) m -> p kk i m", p=P, i=2)
                a2t_re = a2t_ap.rearrange("(kk i p) m -> p kk i m", p=P, i=2)

            hwloop = repeat > 1 and TUNE["hwloop"]
            if hwloop:
                # hardware loop: x fully loaded upfront, body emitted once
                for g in range(KK // XG):
                    nc.gpsimd.dma_start(
                        x_t[:, g * XG : (g + 1) * XG], x_re[:, g * XG : (g + 1) * XG]
                    )
                rep_iter = [0]
                loop_cm = tc.For_i(0, repeat)
                loop_cm.__enter__()
            else:
                rep_iter = list(range(repeat))
            for _rep in rep_iter:
                # acc index: mat*4 + db*2 + mc
                accs = [
                    acc_pool.tile([P, 512], F32, tag="acc", name=f"acc{i}")
                    for i in range(2 * DBLK * MC)
                ]
                diag = TUNE["diag"]
                for kk in range(KK):
                    if not hwloop and _rep == 0 and kk % XG == 0:
                        nc.gpsimd.dma_start(
                            x_t[:, kk : kk + XG], x_re[:, kk : kk + XG]
                        )
                    ch = a_pool.tile([P, 2, M_LOC], FP8, tag="achunk")
                    ch2 = a_pool.tile([P, 2, M_LOC], FP8, tag="achunk")
                    if diag == "nodma":
                        # 1/32 of the bytes: keeps the dep structure, removes
                        # the DMA load so PE-only time is visible
                        nc.sync.dma_start(ch[:, :, :32], at_re[:, kk, :, :32])
                        nc.scalar.dma_start(ch2[:, :, :32], a2t_re[:, kk, :, :32])
                    else:
                        # two HWDGE queues in parallel: at on sync, a2t on
                        # scalar (each ~8 MB/rep)
                        nc.sync.dma_start(ch[:], at_re[:, kk])
                        nc.scalar.dma_start(ch2[:], a2t_re[:, kk])
                    if diag == "nomm" and kk > 0:
                        continue
                    stop_kk = 0 if diag == "nomm" else KK - 1
                    for db in range(DBLK):
                        for mat, c_ch in ((0, ch), (1, ch2)):
                            for mc in range(MC):
                                nc.tensor.matmul(
                                    accs[mat * 4 + db * MC + mc][:],
                                    x_t[:, kk, :, db * P : (db + 1) * P],
                                    c_ch[:, :, mc * 512 : (mc + 1) * 512],
                                    start=(kk == 0),
                                    stop=(kk == stop_kk),
                                    perf_mode=DR,
                                )
                for mat in range(2):
                    for db in range(DBLK):
                        for mc in range(MC):
                            ot = o_pool.tile([P, 512], ODT, tag="outt")
                            if (db * MC + mc) % 2 == 0:
                                nc.vector.tensor_scalar_add(
                                    ot[:], accs[mat * 4 + db * MC + mc][:],
                                    c_t[:, db : db + 1],
                                )
                            else:
                                nc.scalar.add(
                                    ot[:], accs[mat * 4 + db * MC + mc][:],
                                    c_t[:, db : db + 1],
                                )
                            nc.scalar.dma_start(
                                out_ap[
                                    mat * D + db * P : mat * D + (db + 1) * P,
                                    mc * 512 : (mc + 1) * 512,
                                ],
                                ot[:],
                            )
            if hwloop:
                loop_cm.__exit__(None, None, None)

    nc.compile()
    return nc


def _build(mode, repeat=1):
    if mode == "fp8v2":
        return _build_fp8v2(repeat)
    if mode == "fp8drc":
        return _build_fp8drc(repeat)
    if mode == "fp8drb":
        return _build_fp8drb(repeat)
    if mode == "fp8dr":
        return _build_fp8dr(repeat)
    if mode == "bf16t":
        return _build_bf16t(repeat)
    raise ValueError(f"unknown mode {mode}")


def _get_nc(mode, repeat=1):
    key = (mode, repeat, tuple(sorted(TUNE.items())))
    if key not in _cache:
        _cache[key] = _build(mode, repeat)
    return _cache[key]


def make_in_maps(x, adj_t, adj_t2, mode=MODE):
    import ml_dtypes

    x = np.ascontiguousarray(np.asarray(x, dtype=np.float32))
    adj_t = np.asarray(adj_t, dtype=np.float32)
    adj_t2 = np.asarray(adj_t2, dtype=np.float32)
    if mode == "fp8v2":
        e4 = ml_dtypes.float8_e4m3
        # x_v2[p, kk, i, d] = xq[kk*256 + i*128 + p, d]
        xq = np.ascontiguousarray(
            x.astype(e4).reshape(KK, 2, P, D).transpose(2, 0, 1, 3)
        )
        c_row = (0.5 * x.sum(0, dtype=np.float64)).astype(np.float32)
        c = np.ascontiguousarray(c_row.reshape(2, P).T)  # [P, 2] d-blocks

        def prep(adj, sl):
            # A_v2[p, kk, i, m] = fp8(adj[sl][m, kk*256+i*128+p] - 0.5)
            a = (adj[sl] - 0.5).astype(e4)  # [M_LOC, N]
            return np.ascontiguousarray(
                a.reshape(M_LOC, KK, 2, P).transpose(3, 1, 2, 0)
            )

        return [
            {
                "at": prep(adj_t, slice(cid * M_LOC, (cid + 1) * M_LOC)),
                "a2t": prep(adj_t2, slice(cid * M_LOC, (cid + 1) * M_LOC)),
                "x": xq,
                "c": c,
            }
            for cid in range(CORES)
        ]
    if mode in ("fp8dr", "fp8drb", "fp8drc"):
        e4 = ml_dtypes.float8_e4m3
        xq = x.astype(e4)
        c_row = (0.5 * x.sum(0, dtype=np.float64)).astype(np.float32)
        if mode in ("fp8drb", "fp8drc"):
            c = np.ascontiguousarray(c_row.reshape(2, P).T)  # [P, 2] d-blocks
        else:
            c = np.ascontiguousarray(np.broadcast_to(c_row, (P, D)))
        def prep(adj, sl):
            at = np.ascontiguousarray((adj[sl] - 0.5).astype(e4).T)  # [N, M_LOC]
            if mode == "fp8drc" and TUNE["il"]:
                # reorder k-rows (kk, i, p) -> (kk, p, i) so each partition's
                # DoubleRow pair is contiguous in DRAM
                at = np.ascontiguousarray(
                    at.reshape(KK, 2, P, M_LOC).swapaxes(1, 2).reshape(N, M_LOC)
                )
            return at

        maps = []
        for cid in range(CORES):
            sl = slice(cid * M_LOC, (cid + 1) * M_LOC)
            maps.append(
                {
                    "at": prep(adj_t, sl),
                    "a2t": prep(adj_t2, sl),
                    "x": xq,
                    "c": c,
                }
            )
        return maps
    bf = ml_dtypes.bfloat16
    xb = x.astype(bf)
    ab = adj_t.astype(bf)
    a2b = adj_t2.astype(bf)
    return [
        {
            "a": ab[c * M_LOC : (c + 1) * M_LOC],
            "a2": a2b[c * M_LOC : (c + 1) * M_LOC],
            "x": xb,
        }
        for c in range(CORES)
    ]


def gather_out(results, mode=MODE):
    if mode in ("fp8drb", "fp8drc", "fp8v2"):
        return np.concatenate(
            [np.ascontiguousarray(r["out"].T).astype(np.float32) for r in results],
            axis=0,
        )
    return np.concatenate([r["out"] for r in results], axis=0)


def kernel(x, adj_t, adj_t2):
    from concourse.bass_utils import run_bass_kernel_spmd

    nc = _get_nc(MODE)
    in_maps = make_in_maps(x, adj_t, adj_t2, MODE)
    res = run_bass_kernel_spmd(nc, in_maps, core_ids=list(range(CORES)))
    return gather_out(res.results, MODE)



# revision 37
# speedup vs baseline: 1.6384x; 1.1966x over previous
"""H2GCN neighborhood aggregation on 8 Trainium2 NeuronCores.

Computes concat([adj_t @ x, adj_t2 @ x], axis=1) for
adj_t/adj_t2: [8192, 8192] f32, x: [8192, 256] f32.

Sharding: row-shard adj_t/adj_t2 (1024 rows per core), replicate x,
each core produces its [1024, 512] slice of the output.

fp8v2 mode (default): adjacency is centered (a - 0.5) and quantized to
fp8 e4m3 on host, x quantized to e4m3, and the rank-1 term
0.5 * colsum(x) is carried exactly in f32 and added after accumulation.
Matmuls run in MatmulPerfMode.DoubleRow (256 contraction rows per
instruction, 2x PE throughput); measured ~153 TFLOP/s/core, ~98% of the
fp8 PE peak. Adjacency HBM traffic is halved vs bf16 so DMA (~41 us)
hides fully under the PE stream (~56 us). Measured rel err vs the f32
reference: 1.4e-2 (gate 2e-2).

Per-core dataflow (fp8v2):
  - host packs the adjacency slice p-major [128p, 32kk, 2, 1024m] e4m3
    (64 KB contiguous per partition) and x as [128p, 32kk, 2, 256d], so
    every DMA descriptor is a single >=2KB contiguous run.
  - adjacency streams in CKK-kk chunks on the sync HWDGE queue; x lands
    in staged per-group tiles (first tiny group leads the sync ring so
    the first matmul un-gates at ~2.5us; tile-granular dep tracking
    would otherwise gate every matmul on the last x DMA). chunk k's
    4*CKK DoubleRow matmuls accumulate out^T blocks [128d, 512m] in the
    8 PSUM banks (both matrices resident, so the PE never drains
    between matrices).
  - a few throwaway matmuls at t=0 lift the PE HAM clock gate (cold PE
    runs at 1.2 GHz for ~3.4 us) while the first DMAs are in flight.
  - mat0's first / mat1's last chunk are split into single-kk pieces to
    shorten the DMA-gated head and the accumulate->copy->store tail.
  - DVE/Act add the f32 colsum bias during the PSUM -> SBUF bf16 copy;
    stores are per-(mat, d-block) [128, 1024] bf16 rows, the very last
    on the otherwise-idle sync queue. Host un-transposes and upcasts.
"""

import numpy as np

N = 8192
D = 256
CORES = 8
P = 128
M_LOC = N // CORES  # 1024 rows of each adjacency matrix per core
MB = M_LOC // P  # 8 output row-blocks per core
KB = N // P  # 64 contraction blocks
KK = N // (2 * P)  # 32 k-pair blocks (DoubleRow consumes 256 rows/step)
GRP = 8  # k-blocks per transpose/copy group (one PSUM bank), bf16t mode
N_GRP = KB // GRP  # 8

MODE = "fp8v2"  # "fp8v2" / "fp8drb" / "fp8dr" (e4m3 DoubleRow) / "bf16t"

TUNE = dict(
    a_bufs=12,  # fp8 chunk slots in flight
    xg=4,  # kk-blocks per x-load DMA
    chunk_kk=1,  # kk-pairs per adjacency chunk DMA
    out_bf16=True,  # store out as bf16, host upcasts
    diag="none",  # "nodma" (tiny chunk DMAs) / "nomm" (single matmul per acc)
    hwloop=True,  # use tc.For_i for the repeat loop (timing NEFFs only)
    il=True,  # host pair-interleaves adjacency rows (2KB contiguous/partition)
    v2_ckk=4,  # kk-pairs per adjacency chunk DMA (v2)
    v2_xg=8,  # kk-pairs per x-load DMA (v2)
    v2_abufs=8,  # adjacency chunk slots in flight (v2)
    v2_warm=14,  # dummy 32-col matmuls at t=0 to lift the HAM clock gate
    # (sized so PE activity is continuous from ~0.3us until the first
    # DMA-gated real matmul at ~2.3us — an idle gap would let the HAM's
    # free-running idle window re-throttle the clock, while surplus
    # dummies would delay the first real matmul. 32-col keeps each
    # dummy's LDWEIGHTS at the floor so the worst-case cadence is
    # bounded even if LDW doesn't overlap at cold clock)
    v2_2q=False,  # alternate chunk DMAs across both HWDGE rings
)

_cache = {}


def _build_fp8drb(repeat=1):
    """x-stationary orientation: out^T[d, m] accumulates in PSUM while the
    adjacency chunk streams as the 1024-wide moving operand (512 k-pairs).
    Stationary x blocks are reused across 2 moving streams, so LD_WEIGHTS
    is amortized 4x vs the a-stationary orientation. The colsum correction
    is a per-partition bias add; host un-transposes the [512, 1024] out.
    Output is stored bf16 (host upcasts); x loads stream on the gpsimd
    DMA queue interleaved with the first matrix's chunks."""
    import concourse.bacc as bacc
    import concourse.tile as tile
    import concourse.mybir as mybir

    F32 = mybir.dt.float32
    FP8 = mybir.dt.float8e4
    ODT = mybir.dt.bfloat16 if TUNE["out_bf16"] else F32
    DR = mybir.MatmulPerfMode.DoubleRow

    nc = bacc.Bacc(
        "TRN2",
        target_bir_lowering=False,
        debug=False,
        enable_asserts=False,
        num_devices=CORES,
    )
    at_ap = nc.dram_tensor("at", [N, M_LOC], FP8, kind="ExternalInput").ap()
    a2t_ap = nc.dram_tensor("a2t", [N, M_LOC], FP8, kind="ExternalInput").ap()
    x_ap = nc.dram_tensor("x", [N, D], FP8, kind="ExternalInput").ap()
    c_ap = nc.dram_tensor("c", [P, 2], F32, kind="ExternalInput").ap()
    # out is transposed: [2D, M_LOC]; host transposes back (and upcasts)
    out_ap = nc.dram_tensor("out", [2 * D, M_LOC], ODT, kind="ExternalOutput").ap()

    XG = TUNE["xg"]
    CKK = TUNE["chunk_kk"]  # kk-pairs per adjacency chunk DMA
    DBLK = D // P  # 2 stationary d-blocks
    MC = M_LOC // 512  # 2 moving chunks of 512 columns
    with tile.TileContext(nc) as tc:
        with (
            tc.tile_pool(name="xp", bufs=1) as x_pool,
            tc.tile_pool(name="cp", bufs=1) as c_pool,
            tc.tile_pool(name="ap", bufs=TUNE["a_bufs"]) as a_pool,
            tc.tile_pool(name="op", bufs=2 * DBLK * MC) as o_pool,
            tc.tile_pool(name="pacc", bufs=2 * DBLK * MC, space="PSUM") as acc_pool,
        ):
            c_t = c_pool.tile([P, 2], F32)
            nc.gpsimd.dma_start(c_t[:], c_ap[:])

            x_t = x_pool.tile([P, KK, 2, D], FP8)
            x_re = x_ap.rearrange("(kk i p) d -> p kk i d", p=P, i=2)

            for _rep in range(repeat):
                for mat, src_ap in ((0, at_ap), (1, a2t_ap)):
                    accs = [
                        acc_pool.tile([P, 512], F32, tag="acc", name=f"acc{i}")
                        for i in range(DBLK * MC)
                    ]
                    src_re = src_ap.rearrange("(kk i p) m -> p kk i m", p=P, i=2)
                    for kk0 in range(0, KK, CKK):
                        if _rep == 0 and mat == 0 and kk0 % XG == 0:
                            # x chunk rides the gpsimd queue, just ahead of
                            # the adjacency chunks that need it
                            nc.gpsimd.dma_start(
                                x_t[:, kk0 : kk0 + XG], x_re[:, kk0 : kk0 + XG]
                            )
                        ch = a_pool.tile([P, CKK, 2, M_LOC], FP8, tag="achunk")
                        nc.sync.dma_start(ch[:], src_re[:, kk0 : kk0 + CKK])
                        for j in range(CKK):
                            kk = kk0 + j
                            for db in range(DBLK):
                                for mc in range(MC):
                                    nc.tensor.matmul(
                                        accs[db * MC + mc][:],
                                        x_t[:, kk, :, db * P : (db + 1) * P],
                                        ch[:, j, :, mc * 512 : (mc + 1) * 512],
                                        start=(kk == 0),
                                        stop=(kk == KK - 1),
                                        perf_mode=DR,
                                    )
                    for db in range(DBLK):
                        for mc in range(MC):
                            ot = o_pool.tile([P, 512], ODT, tag="outt")
                            if (db * MC + mc) % 2 == 0:
                                nc.vector.tensor_scalar_add(
                                    ot[:], accs[db * MC + mc][:], c_t[:, db : db + 1]
                                )
                            else:
                                nc.scalar.add(
                                    ot[:], accs[db * MC + mc][:], c_t[:, db : db + 1]
                                )
                            nc.scalar.dma_start(
                                out_ap[
                                    mat * D + db * P : mat * D + (db + 1) * P,
                                    mc * 512 : (mc + 1) * 512,
                                ],
                                ot[:],
                            )

    nc.compile()
    return nc


def _emit_v2_out(nc, o_pool, accs, c_t, out_ap, mat, ODT):
    """PSUM -> SBUF bf16 copies (+colsum bias) and per-d-block out stores."""
    import concourse.mybir as mybir  # noqa: F401

    P_, D_, MC_ = P, D, 2
    for db in range(2):
        ot = o_pool.tile([P_, 2, 512], ODT, tag="outt")
        for mc in range(MC_):
            # one full-width copy per engine: splitting these across both
            # engines halves the per-acc drain but serializes 8 half-copies
            # plus store descriptor-gen on Act and sims 1.5us WORSE overall
            if mc == 0:
                nc.vector.tensor_scalar_add(
                    ot[:, mc], accs[db * MC_ + mc][:], c_t[:, db : db + 1]
                )
            else:
                nc.scalar.add(
                    ot[:, mc], accs[db * MC_ + mc][:], c_t[:, db : db + 1]
                )
        # mat0 stores must stay off sync so mat1's chunk stream is never
        # queued behind them (they're emitted before mat1's chunk loop);
        # mat1's stores ride the by-then-idle sync ring so their HWDGE
        # gens don't serialize with the Act-side tail copies
        rows = slice(mat * D_ + db * P_, mat * D_ + (db + 1) * P_)
        if mat == 1 and db == 1:
            # final store: split the two m-halves across both HWDGE rings
            # so the tail's last transfer is half-length and issues in
            # parallel as each copy lands
            nc.sync.dma_start(out_ap[rows, 0:512], ot[:, 0])
            nc.scalar.dma_start(out_ap[rows, 512:1024], ot[:, 1])
        elif mat == 1:
            nc.sync.dma_start(
                out_ap[rows, :], ot[:].rearrange("p mc m -> p (mc m)")
            )
        else:
            nc.scalar.dma_start(
                out_ap[rows, :], ot[:].rearrange("p mc m -> p (mc m)")
            )


def _build_fp8v2(repeat=1):
    """x-stationary orientation like fp8drb, with p-major DRAM layouts so
    every DMA descriptor is a single >=2KB contiguous run per partition:
      at/a2t: [P, KK, 2, M_LOC] fp8 (64 KB contiguous per partition),
      x:      [P, KK, 2, D]     fp8 (16 KB per partition).
    Queue plan (avoids prefetch-blocking): all adjacency chunks stream on
    sync (SP); x + c ride vector (DVE); PSUM->SBUF copies alternate
    vector/scalar; out stores are per-(mat, db) row-blocks [P, 2D] bf16 on
    scalar. Both matrices' accumulators live in the 8 PSUM banks so mat1
    matmuls start while mat0 drains."""
    import concourse.bacc as bacc
    import concourse.tile as tile
    import concourse.mybir as mybir

    F32 = mybir.dt.float32
    FP8 = mybir.dt.float8e4
    ODT = mybir.dt.bfloat16 if TUNE["out_bf16"] else F32
    DR = mybir.MatmulPerfMode.DoubleRow

    nc = bacc.Bacc(
        "TRN2",
        target_bir_lowering=False,
        debug=False,
        enable_asserts=False,
        num_devices=CORES,
    )
    at_ap = nc.dram_tensor("at", [P, KK, 2, M_LOC], FP8, kind="ExternalInput").ap()
    a2t_ap = nc.dram_tensor("a2t", [P, KK, 2, M_LOC], FP8, kind="ExternalInput").ap()
    x_ap = nc.dram_tensor("x", [P, KK, 2, D], FP8, kind="ExternalInput").ap()
    c_ap = nc.dram_tensor("c", [P, 2], F32, kind="ExternalInput").ap()
    # out is transposed: [2D, M_LOC]; host transposes back (and upcasts)
    out_ap = nc.dram_tensor("out", [2 * D, M_LOC], ODT, kind="ExternalOutput").ap()

    CKK = TUNE["v2_ckk"]  # kk-pairs per adjacency chunk DMA
    XG = TUNE["v2_xg"]  # kk-pairs per x DMA
    DBLK = D // P  # 2 stationary d-blocks
    MC = M_LOC // 512  # 2 moving chunks of 512 columns
    with tile.TileContext(nc) as tc:
        with (
            tc.tile_pool(name="xp", bufs=4) as x_pool,
            tc.tile_pool(name="cp", bufs=1) as c_pool,
            tc.tile_pool(name="hp", bufs=2) as h_pool,
            tc.tile_pool(name="ap", bufs=TUNE["v2_abufs"]) as a_pool,
            tc.tile_pool(name="op", bufs=2 * DBLK) as o_pool,
            tc.tile_pool(name="pacc", bufs=2 * DBLK * MC, space="PSUM") as acc_pool,
        ):
            if TUNE["v2_warm"]:
                # lift the PE HAM clock gate during the DMA ramp: a zeroed
                # fp8 tile feeds throwaway 32-col matmuls into acc bank 0;
                # the real accumulation's start=True reset makes them inert.
                # memset on vector so the gpsimd x-load queue isn't delayed.
                warm_t = c_pool.tile([P, 2, 32], FP8)
                nc.vector.memset(warm_t[:], 0.0)

            # x arrives in staged groups, each its OWN tile: tile-granular
            # dependency tracking would otherwise gate every matmul on the
            # LAST x DMA (~+3.5us on the one-shot critical path). The first
            # tiny group is the very first transfer on the sync HWDGE ring
            # (~0.6us first-byte) so the first matmul un-gates immediately;
            # later groups land well before the PE stream reaches them.
            xgroups = [(0, 2), (2, 8), (10, 11), (21, KK - 21)]
            x_ts = {}
            for gi, (g0, gn) in enumerate(xgroups):
                xt = x_pool.tile([P, gn, 2, D], FP8, name=f"xg{gi}")
                q = nc.sync if gi == 0 else nc.gpsimd
                q.dma_start(xt[:], x_ap[:, g0 : g0 + gn])
                for kk in range(g0, g0 + gn):
                    x_ts[kk] = (xt, kk - g0)
            c_t = c_pool.tile([P, 2], F32)
            nc.gpsimd.dma_start(c_t[:], c_ap[:])

            # chunk plans: CKK-sized chunks; mat0's first block split fine so
            # the first matmul isn't gated on a large DMA, mat1's last block
            # split fine so the tail chain is short
            def mk_plan(head_split, tail_split):
                plan, kk0 = [], 0
                while kk0 < KK:
                    ck = min(CKK, KK - kk0)
                    fine = (head_split and kk0 == 0) or (
                        tail_split and kk0 + ck >= KK
                    )
                    if fine and ck > 1:
                        plan.extend((kk0 + j, 1) for j in range(ck))
                    else:
                        plan.append((kk0, ck))
                    kk0 += ck
                return plan

            if TUNE["v2_warm"]:
                warm_acc = acc_pool.tile([P, 512], F32, tag="acc", name="warm")
                for w in range(TUNE["v2_warm"]):
                    nc.tensor.matmul(
                        warm_acc[:32, :32],
                        warm_t[:],
                        warm_t[:],
                        start=True,
                        stop=True,
                        perf_mode=DR,
                        skip_group_check=True,
                    )

            diag = TUNE["diag"]
            two_q = TUNE["v2_2q"]
            for _rep in range(repeat):
                all_accs = {}
                qi = 0
                for mat, src_ap in ((0, at_ap), (1, a2t_ap)):
                    plan = mk_plan(head_split=(_rep == 0 and mat == 0), tail_split=(mat == 1))
                    accs = [
                        acc_pool.tile([P, 512], F32, tag="acc", name=f"acc{mat}_{i}")
                        for i in range(DBLK * MC)
                    ]
                    all_accs[mat] = accs
                    head0 = _rep == 0 and mat == 0
                    for kk0, ck in plan:
                        if head0 and kk0 == 0 and diag == "none":
                            # split chunk0 into per-mc-half tiles on BOTH
                            # HWDGE rings: the mc0 half (364ns, leading the
                            # otherwise-empty scalar ring) un-gates the
                            # first matmuls while the mc1 half + xg0 land
                            # on sync. Separate tiles, not slices — dep
                            # tracking is tile-granular.
                            cha = h_pool.tile([P, 1, 2, 512], FP8, tag="ch0")
                            chb = h_pool.tile([P, 1, 2, 512], FP8, tag="ch0")
                            nc.scalar.dma_start(cha[:], src_ap[:, 0:1, :, 0:512])
                            nc.sync.dma_start(chb[:], src_ap[:, 0:1, :, 512:1024])
                            xt, xj = x_ts[0]
                            for db in range(DBLK):
                                for mc, c_ch in ((0, cha), (1, chb)):
                                    nc.tensor.matmul(
                                        accs[db * MC + mc][:],
                                        xt[:, xj, :, db * P : (db + 1) * P],
                                        c_ch[:, 0],
                                        start=True,
                                        stop=False,
                                        perf_mode=DR,
                                    )
                            continue
                        ch = a_pool.tile([P, CKK, 2, M_LOC], FP8, tag="achunk")
                        q = nc.scalar if (two_q and qi % 2) else nc.sync
                        qi += 1
                        if diag == "nodma":
                            # 1/32 of the bytes: keeps the dep structure,
                            # removes the DMA load so PE-only time shows
                            q.dma_start(
                                ch[:, :ck, :, :32], src_ap[:, kk0 : kk0 + ck, :, :32]
                            )
                        else:
                            q.dma_start(
                                ch[:, :ck], src_ap[:, kk0 : kk0 + ck]
                            )
                        for j in range(ck):
                            kk = kk0 + j
                            if diag == "nomm" and kk > 0:
                                continue
                            stop_kk = 0 if diag == "nomm" else KK - 1
                            xt, xj = x_ts[kk]
                            for db in range(DBLK):
                                for mc in range(MC):
                                    nc.tensor.matmul(
                                        accs[db * MC + mc][:],
                                        xt[:, xj, :, db * P : (db + 1) * P],
                                        ch[:, j, :, mc * 512 : (mc + 1) * 512],
                                        start=(kk == 0),
                                        stop=(kk == stop_kk),
                                        perf_mode=DR,
                                    )
                    if not two_q:
                        _emit_v2_out(nc, o_pool, all_accs[mat], c_t, out_ap, mat, ODT)
                if two_q:
                    # copies/stores emitted after both chunk streams so the
                    # scalar ring's chunk dma_starts are never queued behind
                    # mat0-dependent work (dispatch is issue-and-go)
                    for mat in (0, 1):
                        _emit_v2_out(nc, o_pool, all_accs[mat], c_t, out_ap, mat, ODT)

    nc.compile()
    return nc


def _build_fp8dr(repeat=1):
    import concourse.bacc as bacc
    import concourse.tile as tile
    import concourse.mybir as mybir

    F32 = mybir.dt.float32
    FP8 = mybir.dt.float8e4
    DR = mybir.MatmulPerfMode.DoubleRow

    nc = bacc.Bacc(
        "TRN2",
        target_bir_lowering=False,
        debug=False,
        enable_asserts=False,
        num_devices=CORES,
    )
    at_ap = nc.dram_tensor("at", [N, M_LOC], FP8, kind="ExternalInput").ap()
    a2t_ap = nc.dram_tensor("a2t", [N, M_LOC], FP8, kind="ExternalInput").ap()
    x_ap = nc.dram_tensor("x", [N, D], FP8, kind="ExternalInput").ap()
    c_ap = nc.dram_tensor("c", [P, D], F32, kind="ExternalInput").ap()
    out_ap = nc.dram_tensor("out", [M_LOC, 2 * D], F32, kind="ExternalOutput").ap()

    XG = TUNE["xg"]
    with tile.TileContext(nc) as tc:
        with (
            tc.tile_pool(name="xp", bufs=1) as x_pool,
            tc.tile_pool(name="cp", bufs=1) as c_pool,
            tc.tile_pool(name="ap", bufs=TUNE["a_bufs"]) as a_pool,
            tc.tile_pool(name="op", bufs=MB) as o_pool,
            tc.tile_pool(name="pacc", bufs=MB, space="PSUM") as acc_pool,
        ):
            c_t = c_pool.tile([P, D], F32)
            nc.sync.dma_start(c_t[:], c_ap[:])

            x_t = x_pool.tile([P, KK, 2, D], FP8)
            x_re = x_ap.rearrange("(kk i p) d -> p kk i d", p=P, i=2)
            for g in range(KK // XG):
                nc.sync.dma_start(
                    x_t[:, g * XG : (g + 1) * XG], x_re[:, g * XG : (g + 1) * XG]
                )

            out_ts = [
                o_pool.tile([P, 2 * D], F32, tag="outt", name=f"outt{i}")
                for i in range(MB)
            ]
            for _rep in range(repeat):
                for mat, src_ap in ((0, at_ap), (1, a2t_ap)):
                    accs = [
                        acc_pool.tile([P, D], F32, tag="acc", name=f"acc{i}")
                        for i in range(MB)
                    ]
                    src_re = src_ap.rearrange("(kk i p) m -> p kk i m", p=P, i=2)
                    for kk in range(KK):
                        ch = a_pool.tile([P, 2, M_LOC], FP8, tag="achunk")
                        nc.sync.dma_start(ch[:], src_re[:, kk])
                        for mb in range(MB):
                            nc.tensor.matmul(
                                accs[mb][:],
                                ch[:, :, mb * P : (mb + 1) * P],
                                x_t[:, kk],
                                start=(kk == 0),
                                stop=(kk == KK - 1),
                                perf_mode=DR,
                            )
                    for mb in range(MB):
                        nc.vector.tensor_add(
                            out_ts[mb][:, mat * D : (mat + 1) * D],
                            accs[mb][:],
                            c_t[:],
                        )
                for mb in range(MB):
                    nc.sync.dma_start(out_ap[mb * P : (mb + 1) * P, :], out_ts[mb][:])

    nc.compile()
    return nc


def _build_bf16t(repeat=1):
    """Inputs pre-cast to bf16 on host (halves adjacency HBM traffic).
    A column-stripes [1024, 128] are loaded via the HW xbar DMA-transpose
    directly into matmul-ready [128k, 1024m] layout — no PE transposes, no
    PSUM round-trip. 8 PSUM banks hold one accumulator per output row-block."""
    import concourse.bacc as bacc
    import concourse.tile as tile
    import concourse.mybir as mybir

    F32 = mybir.dt.float32
    BF16 = mybir.dt.bfloat16

    nc = bacc.Bacc(
        "TRN2",
        target_bir_lowering=False,
        debug=False,
        enable_asserts=False,
        num_devices=CORES,
    )
    a_ap = nc.dram_tensor("a", [M_LOC, N], BF16, kind="ExternalInput").ap()
    a2_ap = nc.dram_tensor("a2", [M_LOC, N], BF16, kind="ExternalInput").ap()
    x_ap = nc.dram_tensor("x", [N, D], BF16, kind="ExternalInput").ap()
    out_ap = nc.dram_tensor("out", [M_LOC, 2 * D], F32, kind="ExternalOutput").ap()

    with tile.TileContext(nc) as tc:
        with (
            tc.tile_pool(name="xp", bufs=1) as x_pool,
            tc.tile_pool(name="stp", bufs=6) as st_pool,
            tc.tile_pool(name="op", bufs=MB) as o_pool,
            tc.tile_pool(name="pacc", bufs=MB, space="PSUM") as acc_pool,
        ):
            x_t = x_pool.tile([P, KB, D], BF16)
            x_re = x_ap.rearrange("(j p) d -> p j d", p=P)
            for g in range(N_GRP):
                nc.sync.dma_start(
                    x_t[:, g * GRP : (g + 1) * GRP, :],
                    x_re[:, g * GRP : (g + 1) * GRP, :],
                )

            out_ts = [
                o_pool.tile([P, 2 * D], F32, tag="outt", name=f"outt{i}")
                for i in range(MB)
            ]
            for _rep in range(repeat):
                for mat, src_ap in ((0, a_ap), (1, a2_ap)):
                    accs = [
                        acc_pool.tile([P, D], F32, tag="acc", name=f"acc{i}")
                        for i in range(MB)
                    ]
                    for k in range(KB):
                        st = st_pool.tile([P, M_LOC], BF16, tag="stripe")
                        nc.sync.dma_start_transpose(
                            st[:], src_ap[:, k * P : (k + 1) * P]
                        )
                        for mb in range(MB):
                            nc.tensor.matmul(
                                accs[mb][:],
                                st[:, mb * P : (mb + 1) * P],
                                x_t[:, k, :],
                                start=(k == 0),
                                stop=(k == KB - 1),
                            )
                    for mb in range(MB):
                        if mb % 2 == 0:
                            nc.vector.tensor_copy(
                                out_ts[mb][:, mat * D : (mat + 1) * D], accs[mb][:]
                            )
                        else:
                            nc.scalar.copy(
                                out_ts[mb][:, mat * D : (mat + 1) * D], accs[mb][:]
                            )
                for mb in range(MB):
                    nc.sync.dma_start(out_ap[mb * P : (mb + 1) * P, :], out_ts[mb][:])

    nc.compile()
    return nc


def _build_fp8drc(repeat=1):
    """Like fp8drb but each stationary x(kk, db) load feeds BOTH matrices'
    moving streams (4x 512-pair streams per LDWEIGHTS, 64 loads total).
    All 8 PSUM banks hold the two matrices' accumulators simultaneously."""
    import concourse.bacc as bacc
    import concourse.tile as tile
    import concourse.mybir as mybir

    F32 = mybir.dt.float32
    FP8 = mybir.dt.float8e4
    ODT = mybir.dt.bfloat16 if TUNE["out_bf16"] else F32
    DR = mybir.MatmulPerfMode.DoubleRow

    nc = bacc.Bacc(
        "TRN2",
        target_bir_lowering=False,
        debug=False,
        enable_asserts=False,
        num_devices=CORES,
    )
    at_ap = nc.dram_tensor("at", [N, M_LOC], FP8, kind="ExternalInput").ap()
    a2t_ap = nc.dram_tensor("a2t", [N, M_LOC], FP8, kind="ExternalInput").ap()
    x_ap = nc.dram_tensor("x", [N, D], FP8, kind="ExternalInput").ap()
    c_ap = nc.dram_tensor("c", [P, 2], F32, kind="ExternalInput").ap()
    out_ap = nc.dram_tensor("out", [2 * D, M_LOC], ODT, kind="ExternalOutput").ap()

    XG = TUNE["xg"]
    DBLK = D // P  # 2 stationary d-blocks
    MC = M_LOC // 512  # 2 moving chunks of 512 columns
    with tile.TileContext(nc) as tc:
        with (
            tc.tile_pool(name="xp", bufs=1) as x_pool,
            tc.tile_pool(name="cp", bufs=1) as c_pool,
            tc.tile_pool(name="ap", bufs=TUNE["a_bufs"]) as a_pool,
            tc.tile_pool(name="op", bufs=2 * DBLK * MC) as o_pool,
            tc.tile_pool(name="pacc", bufs=2 * DBLK * MC, space="PSUM") as acc_pool,
        ):
            c_t = c_pool.tile([P, 2], F32)
            nc.gpsimd.dma_start(c_t[:], c_ap[:])

            x_t = x_pool.tile([P, KK, 2, D], FP8)
            x_re = x_ap.rearrange("(kk i p) d -> p kk i d", p=P, i=2)

            if TUNE["il"]:
                # host stored rows as (kk, p, i): per partition the k-pair is
                # one contiguous 2048 B run
                at_re = at_ap.rearrange("(kk p i) m -> p kk i m", p=P, i=2)
                a2t_re = a2t_ap.rearrange("(kk p i) m -> p kk i m", p=P, i=2)
            else:
                at_re = at_ap.rearrange("(kk i p) m -> p kk i m", p=P, i=2)
                a2t_re = a2t_ap.rearrange("(kk i p) m -> p kk i m", p=P, i=2)

            hwloop = repeat > 1 and TUNE["hwloop"]
            if hwloop:
                # hardware loop: x fully loaded upfront, body emitted once
                for g in range(KK // XG):
                    nc.gpsimd.dma_start(
                        x_t[:, g * XG : (g + 1) * XG], x_re[:, g * XG : (g + 1) * XG]
                    )
                rep_iter = [0]
                loop_cm = tc.For_i(0, repeat)
                loop_cm.__enter__()
            else:
                rep_iter = list(range(repeat))
            for _rep in rep_iter:
                # acc index: mat*4 + db*2 + mc
                accs = [
                    acc_pool.tile([P, 512], F32, tag="acc", name=f"acc{i}")
                    for i in range(2 * DBLK * MC)
                ]
                diag = TUNE["diag"]
                for kk in range(KK):
                    if not hwloop and _rep == 0 and kk % XG == 0:
                        nc.gpsimd.dma_start(
                            x_t[:, kk : kk + XG], x_re[:, kk : kk + XG]
                        )
                    ch = a_pool.tile([P, 2, M_LOC], FP8, tag="achunk")
                    ch2 = a_pool.tile([P, 2, M_LOC], FP8, tag="achunk")
                    if diag == "nodma":
                        # 1/32 of the bytes: keeps the dep structure, removes
                        # the DMA load so PE-only time is visible
                        nc.sync.dma_start(ch[:, :, :32], at_re[:, kk, :, :32])
                        nc.scalar.dma_start(ch2[:, :, :32], a2t_re[:, kk, :, :32])
                    else:
                        # two HWDGE queues in parallel: at on sync, a2t on
                        # scalar (each ~8 MB/rep)
                        nc.sync.dma_start(ch[:], at_re[:, kk])
                        nc.scalar.dma_start(ch2[:], a2t_re[:, kk])
                    if diag == "nomm" and kk > 0:
                        continue
                    stop_kk = 0 if diag == "nomm" else KK - 1
                    for db in range(DBLK):
                        for mat, c_ch in ((0, ch), (1, ch2)):
                            for mc in range(MC):
                                nc.tensor.matmul(
                                    accs[mat * 4 + db * MC + mc][:],
                                    x_t[:, kk, :, db * P : (db + 1) * P],
                                    c_ch[:, :, mc * 512 : (mc + 1) * 512],
                                    start=(kk == 0),
                                    stop=(kk == stop_kk),
                                    perf_mode=DR,
                                )
                for mat in range(2):
                    for db in range(DBLK):
                        for mc in range(MC):
                            ot = o_pool.tile([P, 512], ODT, tag="outt")
                            if (db * MC + mc) % 2 == 0:
                                nc.vector.tensor_scalar_add(
                                    ot[:], accs[mat * 4 + db * MC + mc][:],
                                    c_t[:, db : db + 1],
                                )
                            else:
                                nc.scalar.add(
                                    ot[:], accs[mat * 4 + db * MC + mc][:],
                                    c_t[:, db : db + 1],
                                )
                            nc.scalar.dma_start(
                                out_ap[
                                    mat * D + db * P : mat * D + (db + 1) * P,
                                    mc * 512 : (mc + 1) * 512,
                                ],
                                ot[:],
                            )
            if hwloop:
                loop_cm.__exit__(None, None, None)

    nc.compile()
    return nc


def _build(mode, repeat=1):
    if mode == "fp8v2":
        return _build_fp8v2(repeat)
    if mode == "fp8drc":
        return _build_fp8drc(repeat)
    if mode == "fp8drb":
        return _build_fp8drb(repeat)
    if mode == "fp8dr":
        return _build_fp8dr(repeat)
    if mode == "bf16t":
        return _build_bf16t(repeat)
    raise ValueError(f"unknown mode {mode}")


def _get_nc(mode, repeat=1):
    key = (mode, repeat, tuple(sorted(TUNE.items())))
    if key not in _cache:
        _cache[key] = _build(mode, repeat)
    return _cache[key]


def make_in_maps(x, adj_t, adj_t2, mode=MODE):
    import ml_dtypes

    x = np.ascontiguousarray(np.asarray(x, dtype=np.float32))
    adj_t = np.asarray(adj_t, dtype=np.float32)
    adj_t2 = np.asarray(adj_t2, dtype=np.float32)
    if mode == "fp8v2":
        e4 = ml_dtypes.float8_e4m3
        # x_v2[p, kk, i, d] = xq[kk*256 + i*128 + p, d]
        xq = np.ascontiguousarray(
            x.astype(e4).reshape(KK, 2, P, D).transpose(2, 0, 1, 3)
        )
        c_row = (0.5 * x.sum(0, dtype=np.float64)).astype(np.float32)
        c = np.ascontiguousarray(c_row.reshape(2, P).T)  # [P, 2] d-blocks

        def prep(adj, sl):
            # A_v2[p, kk, i, m] = fp8(adj[sl][m, kk*256+i*128+p] - 0.5)
            a = (adj[sl] - 0.5).astype(e4)  # [M_LOC, N]
            return np.ascontiguousarray(
                a.reshape(M_LOC, KK, 2, P).transpose(3, 1, 2, 0)
            )

        return [
            {
                "at": prep(adj_t, slice(cid * M_LOC, (cid + 1) * M_LOC)),
                "a2t": prep(adj_t2, slice(cid * M_LOC, (cid + 1) * M_LOC)),
                "x": xq,
                "c": c,
            }
            for cid in range(CORES)
        ]
    if mode in ("fp8dr", "fp8drb", "fp8drc"):
        e4 = ml_dtypes.float8_e4m3
        xq = x.astype(e4)
        c_row = (0.5 * x.sum(0, dtype=np.float64)).astype(np.float32)
        if mode in ("fp8drb", "fp8drc"):
            c = np.ascontiguousarray(c_row.reshape(2, P).T)  # [P, 2] d-blocks
        else:
            c = np.ascontiguousarray(np.broadcast_to(c_row, (P, D)))
        def prep(adj, sl):
            at = np.ascontiguousarray((adj[sl] - 0.5).astype(e4).T)  # [N, M_LOC]
            if mode == "fp8drc" and TUNE["il"]:
                # reorder k-rows (kk, i, p) -> (kk, p, i) so each partition's
                # DoubleRow pair is contiguous in DRAM
                at = np.ascontiguousarray(
                    at.reshape(KK, 2, P, M_LOC).swapaxes(1, 2).reshape(N, M_LOC)
                )
            return at

        maps = []
        for cid in range(CORES):
            sl = slice(cid * M_LOC, (cid + 1) * M_LOC)
            maps.append(
                {
                    "at": prep(adj_t, sl),
                    "a2t": prep(adj_t2, sl),
                    "x": xq,
                    "c": c,
                }
            )
        return maps
    bf = ml_dtypes.bfloat16
    xb = x.astype(bf)
    ab = adj_t.astype(bf)
    a2b = adj_t2.astype(bf)
    return [
        {
            "a": ab[c * M_LOC : (c + 1) * M_LOC],
            "a2": a2b[c * M_LOC : (c + 1) * M_LOC],
            "x": xb,
        }
        for c in range(CORES)
    ]


def gather_out(results, mode=MODE):
    if mode in ("fp8drb", "fp8drc", "fp8v2"):
        return np.concatenate(
            [np.ascontiguousarray(r["out"].T).astype(np.float32) for r in results],
            axis=0,
        )
    return np.concatenate([r["out"] for r in results], axis=0)


def kernel(x, adj_t, adj_t2):
    from concourse.bass_utils import run_bass_kernel_spmd

    nc = _get_nc(MODE)
    in_maps = make_in_maps(x, adj_t, adj_t2, MODE)
    res = run_bass_kernel_spmd(nc, in_maps, core_ids=list(range(CORES)))
    return gather_out(res.results, MODE)

